# revision 34
# baseline (speedup 1.0000x reference)
"""CvT attention block (depthwise conv proj + BN + talking-heads attention) on 8 trn2 cores.

Sharding: data-parallel over batch (16 batches -> 2 per core).

BN stats: only V needs the cross-core AllReduce. A per-core K mean error is
softmax-shift-invariant (constant across k for each (g,l), even through the
pre_sm head mix); a Q mean error perturbs logits by eps.k_j, incoherent over k;
q/k var errors from 6272/1568 local samples are ~1-2% scale noise that washes
out. V's mean error is channel-coherent and survives the 1/784 attention
average (measured 0.54 rel err with local V stats), so V stats are AllReduced.

Schedule (the previous version stalled ~40us at a late 6x192 AllReduce and then
ran ~70us at K=4/8 because HAM re-throttled the idle PE):
  - conv order v, k, q; the V-stats AllReduce (192x2) launches after the v
    convs (~25us in) and lands during the q convs -- zero PE stall, PE stays
    warm into the attention phase.
  - diag matrices are host-precomputed and DMA'd (frees ScalarE, removes the
    diag-build -> first-conv dependency).
  - input staging is split across 5 DMA rings (sync/scalar/vector/tensor/
    gpsimd) in conv-consumption order; xpad tiles get border-only memsets
    (the interior is fully overwritten by the DMA).
  - A=scale*rsqrt(var+eps) uses batched Ln-then-Exp (Ln and Exp live in
    different ACT table sets; interleaving them cost 11 x 1.6us table loads).

Layouts/folds (unchanged from previous version):
  - host passes inputs channel-major bf16 [b, c, h*w]; depthwise conv runs as
    9 diagonal-matmul taps, taps-outer so consecutive matmuls share one
    LDWEIGHTS per diag.
  - BN folds into the pointwise weights; bias via appended ones-row.
  - pre-softmax talking heads fold into K's weights; post-softmax talking
    heads AND w_out fold into V's weights (193-wide vw per head, col 192 = Z).
  - scores use fp8e4 DoubleRow (K=192 in one pass); sqrt(8)/8 on Q and
    1/sqrt(8) on K balances fp8 ranges.
  - qt is zero-padded to 3200 cols so every scores/AV tile is a full 128/512
    shape (no FD<128 DoubleRow penalty on the ragged 64-tail).
  - scores psum tiles are [112,3,512] (3 PSUM banks); one EXP per 3 taps
    (N=1536) instead of 7 per-bank EXPs -- ACT pays (N+352)/1.2ns per op, so
    fewer, larger EXPs cut ~50us of ScalarE overhead.
"""

import os
import sys
import functools

sys.path.insert(0, "/opt/trn_rl_repo")
os.environ.setdefault("MYCRO_LOCAL_CACHE", "1")

import numpy as np
import ml_dtypes

import concourse.bass as bass
import concourse.mybir as mybir
import concourse.tile as tile
from concourse import bacc
from concourse.bass_utils import run_bass_kernel_spmd

F32 = mybir.dt.float32
BF16 = mybir.dt.bfloat16
FP8 = mybir.dt.float8e4
DR = mybir.MatmulPerfMode.DoubleRow
QK_BAL = 0.35355339059327373   # sqrt(8)/8; applied to both Q and K folds
AF = mybir.ActivationFunctionType
ALU = mybir.AluOpType
AX = mybir.AxisListType

N_CORES = 8
BPC = 2                      # batches per core
C = 192                      # channels
HD = 192                     # num_heads * head_ch
NH = 3
LQ = 3136                    # 56*56
LQP = 3200                   # padded to 25*128 (tail cols are zeros)
LK = 784                     # 28*28
PADW = 58
PADN = PADW * PADW           # 3364
EPS = 1e-5

CT = [(0, 128), (128, 64)]   # channel tiles (partition dim)
KT112 = [(i * 112, 112) for i in range(7)]                # k_pos tiles
LCH = [(i * 512, 512) for i in range(6)] + [(3072, 128)]  # l chunks (padded)

last_results = None


def _emit(tc, nc, io, n_cores=N_CORES, mock_cc=False):
    (xq, xkv, diag_d, dwv_d, vecs_d, pwq_d, pwk_d, pwvT_d, wout_d, sm_d, out_d,
     cc_in, cc_out) = io
    MS = bass.MemorySpace

    with tc.tile_pool(name="wpool", bufs=1) as W, \
         tc.tile_pool(name="ypool", bufs=1) as Y:

        # ---------- static loads, spread over the 3 DMA rings ----------
        # Only SP (sync), Activation (scalar) and gpsimd can initiate DMAs,
        # and a DMA occupies its issuing engine's queue for the transfer.
        # Cross-engine deps are position-based (engine completion counters),
        # so each queue carries only what its consumers need, in consumption
        # order (convs run v, k, q):
        #   sync:   diag_v ci0 | xkv b0/b1 ci0 | diag_k | xq b0/b1 ci0 | weights
        #   scalar: diag_v ci1 | xkv b0/b1 ci1 | diag_q | xq b0/b1 ci1
        #   gpsimd: border memsets only, then the cc chain (AllReduce)
        diag = {}
        for p in range(2):  # v-conv runs on DVE from dw columns, no diag
            for ci, (c0, csz) in enumerate(CT):
                diag[(p, ci)] = W.tile([csz, 9, csz], BF16, tag=f"diag{p}{ci}",
                                       name=f"diag{p}{ci}")
        dwv = []
        for ci, (c0, csz) in enumerate(CT):
            dwv.append(W.tile([csz, 9], F32, tag=f"dwv{ci}", name=f"dwv{ci}"))

        # xpad tiles: all 4 (inp, b) pairs live concurrently
        xpad = {}
        for inp, b, ci in [(i, b, ci) for i in range(2) for b in range(BPC)
                           for ci in range(2)]:
            c0, csz = CT[ci]
            xp = W.tile([csz, PADN], BF16, tag=f"xp{inp}{b}{ci}", name=f"xp{inp}{b}{ci}")
            xpad[(inp, b, ci)] = xp

        def stage(inp, b, ci, eng):
            # host pre-pads to [C, 58*58], so staging is one contiguous DMA
            # (the old 8-row chunk DMAs moved 112-byte bursts at ~60 GB/s)
            c0, csz = CT[ci]
            src = xq if inp == 0 else xkv
            eng.dma_start(xpad[(inp, b, ci)][:, :], src.ap()[b, c0:c0 + csz, :])

        nc.scalar.dma_start(dwv[0][:, :], dwv_d.ap()[0:128, :])
        nc.scalar.dma_start(dwv[1][:, :], dwv_d.ap()[128:192, :])
        nc.sync.dma_start(diag[(1, 0)][:, :, :], diag_d[(1, 0)].ap())
        stage(1, 0, 0, nc.sync)
        stage(1, 0, 1, nc.scalar)
        nc.scalar.dma_start(diag[(1, 1)][:, :, :], diag_d[(1, 1)].ap())
        stage(1, 1, 0, nc.sync)
        stage(1, 1, 1, nc.scalar)
        nc.scalar.dma_start(diag[(0, 0)][:, :, :], diag_d[(0, 0)].ap())
        nc.scalar.dma_start(diag[(0, 1)][:, :, :], diag_d[(0, 1)].ap())
        stage(0, 0, 0, nc.sync)
        stage(0, 0, 1, nc.scalar)
        stage(0, 1, 0, nc.sync)
        stage(0, 1, 1, nc.scalar)

        # small weights (needed at fold time) on the sync ring, after staging
        vecs = []
        for ci, (c0, csz) in enumerate(CT):
            t = W.tile([csz, 6], F32, tag=f"vecs{ci}", name=f"vecs{ci}")
            nc.sync.dma_start(t[:, :], vecs_d.ap()[c0:c0 + csz, :])
            vecs.append(t)
        # smbc is host-prebuilt [128, 18] with QK_BAL folded into cols 0-8
        smbc = W.tile([128, 18], F32, tag="smbc")
        nc.sync.dma_start(smbc[:, :], sm_d.ap()[:, :])

        pwq_sb, pwk_sb, pwvT_sb, wout_sb = [], [], [], []
        for ci, (c0, csz) in enumerate(CT):
            for lst, dram, nm in ((pwq_sb, pwq_d, "pwq"), (pwk_sb, pwk_d, "pwk"),
                                  (pwvT_sb, pwvT_d, "pwvT"), (wout_sb, wout_d, "wout")):
                t = W.tile([csz, 192], F32, tag=f"{nm}{ci}", name=f"{nm}{ci}")
                nc.sync.dma_start(t[:, :], dram.ap()[c0:c0 + csz, :])
                lst.append(t)

        # conv outputs (augmented with ones row on tile 2)
        ysz = {0: LQ, 1: LK, 2: LK}
        y = {}
        for b in range(BPC):
            for p in range(3):
                y[(b, p, 0)] = Y.tile([128, ysz[p]], BF16, tag=f"y{b}{p}0", name=f"y{b}{p}0")
                y[(b, p, 1)] = Y.tile([65, ysz[p]], BF16, tag=f"y{b}{p}1", name=f"y{b}{p}1")
                nc.vector.memset(y[(b, p, 1)][64:65, :], 1.0)

        # per-path bn_stats slots: q 14 groups, k/v 4 groups of 6
        slots = {}
        for p, ngrp in ((0, 14), (1, 4), (2, 4)):
            slots[p] = [W.tile([csz, 6 * ngrp], F32, tag=f"sl{p}{ci}", name=f"sl{p}{ci}")
                        for ci, (c0, csz) in enumerate(CT)]
        mv = {p: [W.tile([csz, 2], F32, tag=f"mv{p}{ci}", name=f"mv{p}{ci}")
                  for ci, (c0, csz) in enumerate(CT)] for p in range(3)}
        ccst = [W.tile([csz, 3], F32, tag=f"ccst{ci}", name=f"ccst{ci}")
                for ci, (c0, csz) in enumerate(CT)]
        gst = [W.tile([csz, 2], F32, tag=f"gst{ci}", name=f"gst{ci}")
               for ci, (c0, csz) in enumerate(CT)]

        # phase-2 tiles
        # ab cols: [A_q' 0 | A_k 1 | A_v 2 | mean_q 3 | mean_k 4 | mean_v 5]
        ab = [W.tile([csz, 6], F32, tag=f"ab{ci}", name=f"ab{ci}")
              for ci, (c0, csz) in enumerate(CT)]
        bbf = [W.tile([csz, 3], BF16, tag=f"bbf{ci}", name=f"bbf{ci}")
               for ci, (c0, csz) in enumerate(CT)]
        vep = [W.tile([csz, 3], F32, tag=f"vep{ci}", name=f"vep{ci}")
               for ci, (c0, csz) in enumerate(CT)]
        lt = [W.tile([csz, 3], F32, tag=f"lt{ci}", name=f"lt{ci}")
              for ci, (c0, csz) in enumerate(CT)]
        rstd = [W.tile([csz, 3], F32, tag=f"rstd{ci}", name=f"rstd{ci}")
                for ci, (c0, csz) in enumerate(CT)]
        tmp = [W.tile([csz, 2], F32, tag=f"tmp{ci}", name=f"tmp{ci}")
               for ci, (c0, csz) in enumerate(CT)]
        NTOT_V = float(n_cores * BPC * LK)

        pwqA = [W.tile([128, 192], BF16, tag="pwqA0", name="pwqA0"),
                W.tile([65, 192], BF16, tag="pwqA1", name="pwqA1")]
        pwkA = [W.tile([csz, 192], BF16, tag=f"pwkA{ci}", name=f"pwkA{ci}")
                for ci, (c0, csz) in enumerate(CT)]
        browk = W.tile([1, 192], F32, tag="browk")
        pwvT_bf = [W.tile([csz, 192], BF16, tag=f"pwvTb{ci}", name=f"pwvTb{ci}")
                   for ci, (c0, csz) in enumerate(CT)]
        postvec = W.tile([128, 3], F32, tag="postvec")
        wbar = [W.tile([128, 192], BF16, tag="wbar0", name="wbar0"),
                W.tile([64, 192], BF16, tag="wbar1", name="wbar1")]
        kw, cw = {}, {}
        for g in range(NH):
            kw[(g, 0)] = W.tile([128, 192], BF16, tag=f"kw{g}0", name=f"kw{g}0")
            kw[(g, 1)] = W.tile([65, 192], BF16, tag=f"kw{g}1", name=f"kw{g}1")
            cw[(g, 0)] = W.tile([128, 193], BF16, tag=f"cw{g}0", name=f"cw{g}0")
            cw[(g, 1)] = W.tile([65, 193], BF16, tag=f"cw{g}1", name=f"cw{g}1")

        def ab_chain(paths):
            # A = scale * rsqrt(var+eps), b'' = offset/A - mean; batched Ln
            # pass then batched Exp pass (Ln and Exp are in different ACT
            # table sets -- interleaving would reload tables per op)
            p0, p1 = min(paths), max(paths) + 1
            for ci, (c0, csz) in enumerate(CT):
                for p in paths:
                    if p < 2:  # q, k: local batch stats
                        nc.vector.tensor_scalar(ab[ci][:, 3 + p:4 + p],
                                                mv[p][ci][:, 0:1], 1.0, None, ALU.mult)
                        nc.vector.tensor_scalar(vep[ci][:, p:p + 1], mv[p][ci][:, 1:2],
                                                1.0, EPS, ALU.mult, ALU.add)
                    else:      # v: global stats from the AllReduce
                        inv_n = 1.0 / NTOT_V
                        mean_v = ab[ci][:, 5:6]
                        nc.vector.tensor_scalar(mean_v, gst[ci][:, 0:1], inv_n,
                                                None, ALU.mult)
                        nc.vector.tensor_scalar(tmp[ci][:, 0:1], gst[ci][:, 1:2],
                                                inv_n, EPS, ALU.mult, ALU.add)
                        nc.vector.tensor_scalar(tmp[ci][:, 1:2], mean_v, mean_v,
                                                None, ALU.mult)
                        nc.vector.tensor_tensor(vep[ci][:, 2:3], tmp[ci][:, 0:1],
                                                tmp[ci][:, 1:2], ALU.subtract)
                nc.scalar.activation(lt[ci][:, p0:p1], vep[ci][:, p0:p1], AF.Ln)
            for ci, (c0, csz) in enumerate(CT):
                nc.scalar.activation(rstd[ci][:, p0:p1], lt[ci][:, p0:p1],
                                     AF.Exp, scale=-0.5)
                for p in paths:
                    A = ab[ci][:, p:p + 1]
                    nc.vector.tensor_scalar(A, rstd[ci][:, p:p + 1],
                                            vecs[ci][:, 2 * p:2 * p + 1], None, ALU.mult)
                    recA = tmp[ci][:, 0:1]
                    nc.vector.reciprocal(recA, A)
                    bpp = tmp[ci][:, 1:2]       # b'' = offset*recA - mean
                    nc.vector.scalar_tensor_tensor(bpp, vecs[ci][:, 2 * p + 1:2 * p + 2],
                                                   recA, ab[ci][:, 3 + p:4 + p],
                                                   ALU.mult, ALU.subtract)
                    nc.vector.tensor_scalar(bbf[ci][:, p:p + 1], bpp, 1.0, None, ALU.mult)
                    if p == 0:
                        nc.vector.tensor_scalar(A, A, QK_BAL, None, ALU.mult)

        # ---------- phase 1+2: convs (v on DVE; k, q on PE) + folds ----------
        with tc.tile_pool(name="pconv", bufs=6, space=MS.PSUM) as PCONV, \
             tc.tile_pool(name="prow", bufs=1, space=MS.PSUM) as PROW, \
             tc.tile_pool(name="pcw", bufs=1, space=MS.PSUM) as PCW, \
             tc.tile_pool(name="accv", bufs=2) as ACCV:

            def conv_v_dve(b):
                # stride-2 depthwise conv as 9 shifted multiply-accumulates on
                # the (otherwise idle) VectorE -- frees ~12us of PE and starts
                # the V-stats AllReduce chain without waiting on the PE
                for ci, (c0, csz) in enumerate(CT):
                    xv = xpad[(1, b, ci)].rearrange("p (h th w tw) -> p h th w tw",
                                                    th=2, tw=2, w=29)
                    for kc in range(2):  # DVE free-dim cap is 512 -> 392 chunks
                        acc = ACCV.tile([csz, 14, 28], F32, tag="vacc", name="vacc")
                        t = 0
                        for dy in (0, 1, 2):
                            for dx in (0, 1, 2):
                                h0, th = divmod(28 * kc + dy + 1, 2)
                                w0, tw = divmod(dx + 1, 2)
                                src = xv[0:csz, h0:h0 + 14, th, w0:w0 + 28, tw]
                                if t == 0:
                                    nc.vector.tensor_scalar(acc[:, :, :], src,
                                                            dwv[ci][:, t:t + 1], None, ALU.mult)
                                else:
                                    nc.vector.scalar_tensor_tensor(acc[:, :, :], src,
                                                                   dwv[ci][:, t:t + 1],
                                                                   acc[:, :, :],
                                                                   ALU.mult, ALU.add)
                                t += 1
                        ysl = y[(b, 2, ci)][0:csz, 392 * kc:392 * (kc + 1)]
                        nc.vector.tensor_copy(ysl, acc.rearrange("p h w -> p (h w)"))
                        si = 2 * b + kc
                        nc.vector.bn_stats(slots[2][ci][:, 6 * si:6 * si + 6], ysl)

            def conv_kv(b, p):
                # stride 2 over xkv, psum chunks of 392 (14 output rows);
                # psum->y copies on VectorE (ScalarE is busy staging xq)
                for ci, (c0, csz) in enumerate(CT):
                    xv = xpad[(1, b, ci)].rearrange("p (h th w tw) -> p h th w tw",
                                                    th=2, tw=2, w=29)
                    pss = [PCONV.tile([csz, 392], F32, tag="convps", name="convps")
                           for _ in range(2)]
                    t = 0
                    for dy in (0, 1, 2):
                        for dx in (0, 1, 2):
                            for kc in range(2):
                                h0, th = divmod(28 * kc + dy + 1, 2)
                                w0, tw = divmod(dx + 1, 2)
                                rhs = xv[0:csz, h0:h0 + 14, th, w0:w0 + 28, tw]
                                nc.tensor.matmul(pss[kc][:, :], diag[(p, ci)][:, t, :], rhs,
                                                 start=(t == 0), stop=(t == 8))
                            t += 1
                    for kc in range(2):
                        si = 2 * b + kc
                        ysl = y[(b, p, ci)][0:csz, 392 * kc:392 * (kc + 1)]
                        nc.vector.tensor_copy(ysl, pss[kc][:, :])
                        nc.vector.bn_stats(slots[p][ci][:, 6 * si:6 * si + 6], ysl)

            def conv_q(b):
                # stride 1, psum chunks of 448 (8 output rows); taps outer so
                # runs of 7 matmuls share one diag LDWEIGHTS
                for ci, (c0, csz) in enumerate(CT):
                    xv = xpad[(0, b, ci)].rearrange("p (h w) -> p h w", w=PADW)
                    pss = [PCONV.tile([csz, 448], F32, tag="convps", name="convps")
                           for _ in range(7)]
                    t = 0
                    for dy in (-1, 0, 1):
                        for dx in (-1, 0, 1):
                            for qc in range(7):
                                r0 = 8 * qc + 1 + dy
                                rhs = xv[0:csz, r0:r0 + 8, 1 + dx:57 + dx]
                                nc.tensor.matmul(pss[qc][:, :], diag[(0, ci)][:, t, :], rhs,
                                                 start=(t == 0), stop=(t == 8))
                            t += 1
                    for qc in range(7):
                        si = 7 * b + qc
                        ysl = y[(b, 0, ci)][0:csz, 448 * qc:448 * (qc + 1)]
                        nc.scalar.activation(ysl, pss[qc][:, :], AF.Copy)
                        nc.vector.bn_stats(slots[0][ci][:, 6 * si:6 * si + 6], ysl)

            conv_v_dve(0)
            conv_v_dve(1)

            # v stats -> (sum, sumsq) -> AllReduce, launched under the k/q convs
            NLOC_V = float(BPC * LK)
            for ci, (c0, csz) in enumerate(CT):
                nc.vector.bn_aggr(mv[2][ci][:, 0:2], slots[2][ci][:, 0:24])
                m = mv[2][ci][:, 0:1]
                v = mv[2][ci][:, 1:2]
                nc.vector.tensor_scalar(ccst[ci][:, 0:1], m, NLOC_V, None, ALU.mult)
                nc.vector.tensor_scalar(ccst[ci][:, 2:3], m, m, None, ALU.mult)
                nc.vector.tensor_scalar(ccst[ci][:, 1:2], v, ccst[ci][:, 2:3],
                                        NLOC_V, ALU.add, ALU.mult)
                nc.gpsimd.dma_start(cc_in.ap()[c0:c0 + csz, :], ccst[ci][:, 0:2])
            if mock_cc:
                nc.gpsimd.dma_start(cc_out.ap()[:, :], cc_in.ap()[:, :])
            else:
                nc.gpsimd.collective_compute(
                    "AllReduce", ALU.add, replica_groups=[list(range(n_cores))],
                    ins=[cc_in.ap()], outs=[cc_out.ap()])
            for ci, (c0, csz) in enumerate(CT):
                nc.gpsimd.dma_start(gst[ci][:, :], cc_out.ap()[c0:c0 + csz, :])

            conv_kv(0, 1)
            conv_kv(1, 1)
            for ci in range(2):
                nc.vector.bn_aggr(mv[1][ci][:, 0:2], slots[1][ci][:, 0:24])

            conv_q(0)

            # ----- fold k while the PE runs q convs (local stats only, so
            # the PE's kw bias-row matmul can't wedge on the AllReduce) -----
            ab_chain((1,))

            # k weights + pre_sm folding
            for ci, (c0, csz) in enumerate(CT):
                nc.vector.tensor_scalar(pwkA[ci][:, :], pwk_sb[ci][:, :],
                                        ab[ci][:, 1:2], None, ALU.mult)
            rps = PROW.tile([1, 192], F32, tag="rowps", name="rowps")
            nc.tensor.matmul(rps[:, :], bbf[0][:, 1:2], pwkA[0][:, :], start=True, stop=False)
            nc.tensor.matmul(rps[:, :], bbf[1][:, 1:2], pwkA[1][:, :], start=False, stop=True)
            nc.vector.tensor_copy(browk[:, :], rps[:, :])
            for g in range(NH):
                for h in range(NH):
                    col = 3 * h + g
                    for ci, (c0, csz) in enumerate(CT):
                        nc.vector.tensor_scalar(kw[(g, ci)][0:csz, 64 * h:64 * (h + 1)],
                                                pwkA[ci][:, 64 * h:64 * (h + 1)],
                                                smbc[0:csz, col:col + 1], None, ALU.mult)
                    nc.vector.tensor_scalar(kw[(g, 1)][64:65, 64 * h:64 * (h + 1)],
                                            browk[:, 64 * h:64 * (h + 1)],
                                            smbc[0:1, col:col + 1], None, ALU.mult)

            conv_q(1)

            # ----- fold v (needs the AllReduce, long since landed) -----
            ab_chain((2,))

            # v weights: cw_aug_g with post_sm + w_out folded
            for ci, (c0, csz) in enumerate(CT):
                nc.vector.tensor_copy(pwvT_bf[ci][:, :], pwvT_sb[ci][:, :])
            for g in range(NH):
                nc.vector.tensor_copy(postvec[0:64, g:g + 1], smbc[0:64, 9 + 3 * g:10 + 3 * g])
                nc.vector.tensor_copy(postvec[64:128, g:g + 1], smbc[64:128, 10 + 3 * g:11 + 3 * g])
                nc.vector.tensor_scalar(wbar[0][:, :], wout_sb[0][:, :],
                                        postvec[:, g:g + 1], None, ALU.mult)
                nc.vector.tensor_scalar(wbar[1][:, :], wout_sb[1][:, :],
                                        smbc[0:64, 11 + 3 * g:12 + 3 * g], None, ALU.mult)
                for ci, (c0, csz) in enumerate(CT):
                    ps = PCW.tile([csz, 192], F32, tag="cwps", name="cwps")
                    nc.tensor.matmul(ps[:, :], pwvT_bf[0][:, c0:c0 + csz], wbar[0][:, :],
                                     start=True, stop=False)
                    nc.tensor.matmul(ps[:, :], pwvT_bf[1][:, c0:c0 + csz], wbar[1][:, :],
                                     start=False, stop=True)
                    nc.scalar.activation(cw[(g, ci)][0:csz, 0:192], ps[:, :], AF.Copy,
                                         scale=ab[ci][:, 2:3])
                rps = PROW.tile([1, 192], F32, tag="rowps", name="rowps")
                nc.tensor.matmul(rps[:, :], bbf[0][:, 2:3], cw[(g, 0)][0:128, 0:192],
                                 start=True, stop=False)
                nc.tensor.matmul(rps[:, :], bbf[1][:, 2:3], cw[(g, 1)][0:64, 0:192],
                                 start=False, stop=True)
                nc.vector.tensor_copy(cw[(g, 1)][64:65, 0:192], rps[:, :])
                nc.gpsimd.memset(cw[(g, 0)][:, 192:193], 0.0)
                nc.gpsimd.memset(cw[(g, 1)][0:64, 192:193], 0.0)
                nc.gpsimd.memset(cw[(g, 1)][64:65, 192:193], 1.0)

            for ci in range(2):
                nc.vector.bn_aggr(mv[0][ci][:, 0:2], slots[0][ci][:, 0:84])

        # ---------- phase 3: projections + attention ----------
        with tc.tile_pool(name="qtpool", bufs=2) as QT, \
             tc.tile_pool(name="ktpool", bufs=2) as KTP, \
             tc.tile_pool(name="vwpool", bufs=2) as VWP, \
             tc.tile_pool(name="epool", bufs=2) as EP, \
             tc.tile_pool(name="accpool", bufs=3) as ACC, \
             tc.tile_pool(name="outpool", bufs=3) as OUT, \
             tc.tile_pool(name="rzpool", bufs=8) as RZ, \
             tc.tile_pool(name="stri", bufs=2, space=MS.PSUM) as STRI, \
             tc.tile_pool(name="pav", bufs=2, space=MS.PSUM) as PAV:

            for b in range(BPC):
                # q/k projections land in fp8 [96, 2, L]: channel c = 96*i + p,
                # the layout the DoubleRow score matmul contracts directly.
                kt = {}
                for g in range(NH):
                    kt[g] = KTP.tile([96, 2, LK], FP8, tag=f"kt{g}", name=f"kt{g}")
                    for k0 in (0, 392):
                        for mi in range(2):
                            ps3 = STRI.tile([112, 3, 512], F32, tag="stri", name="stri")
                            ps = ps3[0:96, 0, 0:392]
                            nc.tensor.matmul(ps, kw[(g, 0)][:, 96 * mi:96 * mi + 96],
                                             y[(b, 1, 0)][:, k0:k0 + 392], start=True, stop=False)
                            nc.tensor.matmul(ps, kw[(g, 1)][:, 96 * mi:96 * mi + 96],
                                             y[(b, 1, 1)][:, k0:k0 + 392], start=False, stop=True)
                            nc.scalar.activation(kt[g][:, mi, k0:k0 + 392], ps, AF.Copy)

                if b == 0:
                    # ----- fold q: its vector/scalar chain runs under the kt
                    # projection above (which only needs kw) -----
                    ab_chain((0,))
                    for ci, (c0, csz) in enumerate(CT):
                        nc.vector.tensor_scalar(pwqA[ci][0:csz, :], pwq_sb[ci][:, :],
                                                ab[ci][:, 0:1], None, ALU.mult)
                    rps = STRI.tile([112, 3, 512], F32, tag="stri",
                                    name="stri")[0:1, 0, 0:192]
                    nc.tensor.matmul(rps, bbf[0][:, 0:1], pwqA[0][0:128, :],
                                     start=True, stop=False)
                    nc.tensor.matmul(rps, bbf[1][:, 0:1], pwqA[1][0:64, :],
                                     start=False, stop=True)
                    nc.vector.tensor_copy(pwqA[1][64:65, :], rps)

                vw = {}
                for g in range(NH):
                    vw[g] = VWP.tile([112, 7, 193], BF16, tag=f"vw{g}", name=f"vw{g}")
                    for ki, (k0, ksz) in enumerate(KT112):
                        ps = PAV.tile([128, 193], F32, tag="avps", name="avps")[0:112, :]
                        nc.tensor.matmul(ps, y[(b, 2, 0)][:, k0:k0 + ksz], cw[(g, 0)][:, :],
                                         start=True, stop=False)
                        nc.tensor.matmul(ps, y[(b, 2, 1)][:, k0:k0 + ksz], cw[(g, 1)][:, :],
                                         start=False, stop=True)
                        nc.vector.tensor_copy(vw[g][:, ki, :], ps)

                qt = QT.tile([96, 2, LQP], FP8, tag="qt", name="qt")
                for mi in range(2):
                    nc.gpsimd.memset(qt[:, mi, LQ:LQP], 0.0)
                for l0, lsz in [(i * 512, 512) for i in range(6)] + [(3072, 64)]:
                    for mi in range(2):
                        ps3 = STRI.tile([112, 3, 512], F32, tag="stri", name="stri")
                        ps = ps3[0:96, 0, 0:lsz]
                        nc.tensor.matmul(ps, pwqA[0][:, 96 * mi:96 * mi + 96],
                                         y[(b, 0, 0)][:, l0:l0 + lsz], start=True, stop=False)
                        nc.tensor.matmul(ps, pwqA[1][:, 96 * mi:96 * mi + 96],
                                         y[(b, 0, 1)][:, l0:l0 + lsz], start=False, stop=True)
                        nc.scalar.activation(qt[:, mi, l0:l0 + lsz], ps, AF.Copy)

                for l0, lsz in LCH:
                    ee = {}
                    for g in range(NH):
                        ee[g] = EP.tile([112, 7, 512], BF16, tag=f"e{g}", name=f"e{g}")
                        for g0 in (0, 3, 6):
                            ng = min(3, 7 - g0)
                            ps3 = STRI.tile([112, 3, 512], F32, tag="stri", name="stri")
                            for j in range(ng):
                                k0, ksz = KT112[g0 + j]
                                nc.tensor.matmul(ps3[:, j, 0:lsz], kt[g][:, :, k0:k0 + ksz],
                                                 qt[:, :, l0:l0 + lsz],
                                                 start=True, stop=True, perf_mode=DR)
                            nc.scalar.activation(ee[g][:, g0:g0 + ng, 0:lsz],
                                                 ps3[:, 0:ng, 0:lsz], AF.Exp)

                    for ls in range(0, lsz, 128):
                        acc = ACC.tile([128, 192], F32, tag="acc", name="acc")
                        ot = OUT.tile([128, 192], F32, tag="ot", name="ot")
                        for g in range(NH):
                            ps = PAV.tile([128, 193], F32, tag="avps", name="avps")
                            pv = ps[0:128, 0:193]
                            for ki in range(7):
                                nc.tensor.matmul(pv, ee[g][:, ki, ls:ls + 128],
                                                 vw[g][:, ki, :],
                                                 start=(ki == 0), stop=(ki == 6))
                            r = RZ.tile([128, 1], F32, tag="rz", name="rz")
                            nc.vector.reciprocal(r[:, :], pv[:, 192:193])
                            if g == 0:
                                nc.vector.tensor_scalar(acc[:, :], pv[:, 0:192], r[:, :],
                                                        None, ALU.mult)
                            elif g == 1:
                                nc.vector.scalar_tensor_tensor(acc[:, :], pv[:, 0:192],
                                                               r[:, :], acc[:, :],
                                                               ALU.mult, ALU.add)
                            else:
                                nc.vector.scalar_tensor_tensor(ot[:, :], pv[:, 0:192],
                                                               r[:, :], acc[:, :],
                                                               ALU.mult, ALU.add)
                        lw_out = 64 if l0 + ls == 3072 else 128
                        nc.sync.dma_start(out_d.ap()[b, l0 + ls:l0 + ls + lw_out, :],
                                          ot[0:lw_out, :])


def build(n_cores=N_CORES, mock_cc=False):
    nc = bacc.Bacc("TRN2", target_bir_lowering=False, debug=False, num_devices=n_cores)
    xq = nc.dram_tensor("xq", [BPC, C, PADN], BF16, kind="ExternalInput")
    xkv = nc.dram_tensor("xkv", [BPC, C, PADN], BF16, kind="ExternalInput")
    diag_d = {}
    for p in range(2):
        for ci, (c0, csz) in enumerate(CT):
            diag_d[(p, ci)] = nc.dram_tensor(f"diag{p}{ci}", [csz, 9, csz], BF16,
                                             kind="ExternalInput")
    dwv_d = nc.dram_tensor("dwv", [C, 9], F32, kind="ExternalInput")
    vecs_d = nc.dram_tensor("vecs", [C, 6], F32, kind="ExternalInput")
    pwq_d = nc.dram_tensor("pwq", [C, HD], F32, kind="ExternalInput")
    pwk_d = nc.dram_tensor("pwk", [C, HD], F32, kind="ExternalInput")
    pwvT_d = nc.dram_tensor("pwvT", [HD, C], F32, kind="ExternalInput")
    wout_d = nc.dram_tensor("wout", [HD, C], F32, kind="ExternalInput")
    sm_d = nc.dram_tensor("smm", [128, 18], F32, kind="ExternalInput")
    out_d = nc.dram_tensor("out", [BPC, LQ, C], F32, kind="ExternalOutput")
    cc_in = nc.dram_tensor("cc_in", [C, 2], F32)
    cc_out = nc.dram_tensor("cc_out", [C, 2], F32)

    io = (xq, xkv, diag_d, dwv_d, vecs_d, pwq_d, pwk_d, pwvT_d, wout_d, sm_d,
          out_d, cc_in, cc_out)
    with tile.TileContext(nc) as tc:
        _emit(tc, nc, io, n_cores=n_cores, mock_cc=mock_cc)
    nc.compile()
    return nc


@functools.lru_cache(maxsize=1)
def _built():
    return build()


def make_in_maps(inputs):
    f32c = lambda a: np.ascontiguousarray(np.asarray(a), dtype=np.float32)
    bf16c = lambda a: np.ascontiguousarray(np.asarray(a, dtype=np.float32)
                                           .astype(ml_dtypes.bfloat16))
    def pad_cm(a):  # [16,56,56,C] -> channel-major zero-padded [16, C, 58*58]
        t = np.asarray(a, dtype=np.float32).transpose(0, 3, 1, 2)
        p = np.zeros((16, C, PADW, PADW), dtype=ml_dtypes.bfloat16)
        p[:, :, 1:57, 1:57] = t.astype(ml_dtypes.bfloat16)
        return np.ascontiguousarray(p.reshape(16, C, PADN))

    xq_t = pad_cm(inputs["inputs_q"])
    xkv_t = pad_cm(inputs["inputs_kv"])
    dwt = np.concatenate([np.asarray(inputs[k], dtype=np.float32).reshape(9, C).T
                          for k in ("dw_q", "dw_k", "dw_v")], axis=1)  # [C, 27]
    base = {
        "vecs": f32c(np.stack([np.asarray(inputs[k]) for k in
                               ("scale_q", "offset_q", "scale_k", "offset_k",
                                "scale_v", "offset_v")], axis=1)),
        "smm": f32c(np.broadcast_to(np.concatenate(
            [np.asarray(inputs["pre_sm"]).reshape(-1) * QK_BAL,
             np.asarray(inputs["post_sm"]).reshape(-1)])[None, :], (128, 18))),
        "pwq": f32c(inputs["pw_q"]), "pwk": f32c(inputs["pw_k"]),
        "pwvT": f32c(np.asarray(inputs["pw_v"]).T), "wout": f32c(inputs["w_out"]),
    }
    for p in range(2):
        for ci, (c0, csz) in enumerate(CT):
            d = np.zeros((csz, 9, csz), dtype=ml_dtypes.bfloat16)
            blk = dwt[c0:c0 + csz, 9 * p:9 * p + 9].astype(ml_dtypes.bfloat16)
            d[np.arange(csz)[:, None], np.arange(9)[None, :], np.arange(csz)[:, None]] = blk
            base[f"diag{p}{ci}"] = d
    base["dwv"] = np.ascontiguousarray(dwt[:, 18:27])
    in_maps = []
    for i in range(N_CORES):
        m = dict(base)
        m["xq"] = np.ascontiguousarray(xq_t[BPC * i:BPC * (i + 1)])
        m["xkv"] = np.ascontiguousarray(xkv_t[BPC * i:BPC * (i + 1)])
        in_maps.append(m)
    return in_maps


def kernel(**inputs):
    global last_results
    nc = _built()
    in_maps = make_in_maps(inputs)
    trace = os.environ.get("BASS_KERNEL_TRACE", "0") == "1"
    res = run_bass_kernel_spmd(nc, in_maps, core_ids=list(range(N_CORES)), trace=trace)
    last_results = res
    out = np.concatenate([res.results[i]["out"] for i in range(N_CORES)], axis=0)
    return out.astype(np.float32)


if __name__ == "__main__":
    import reference
    inputs = reference.setup_inputs()
    expected = np.asarray(reference.reference(**inputs))
    actual = kernel(**inputs)
    d = np.abs(actual - expected)
    print(f"absmax={d.max():.3e} scale={np.abs(expected).max():.3e} "
          f"rel={d.max() / np.abs(expected).max():.3e}")


# revision 36
# speedup vs baseline: 1.0698x; 1.0698x over previous
"""CvT attention block (depthwise conv proj + BN + talking-heads attention) on 8 trn2 cores.

Sharding: data-parallel over batch (16 batches -> 2 per core).

BN stats: only V needs the cross-core AllReduce. A per-core K mean error is
softmax-shift-invariant (constant across k for each (g,l), even through the
pre_sm head mix); a Q mean error perturbs logits by eps.k_j, incoherent over k;
q/k var errors from 6272/1568 local samples are ~1-2% scale noise that washes
out. V's mean error is channel-coherent and survives the 1/784 attention
average (measured 0.54 rel err with local V stats), so V stats are AllReduced.

Schedule (the previous version stalled ~40us at a late 6x192 AllReduce and then
ran ~70us at K=4/8 because HAM re-throttled the idle PE):
  - conv order v, k, q; the V-stats AllReduce (192x2) launches after the v
    convs (~25us in) and lands during the q convs -- zero PE stall, PE stays
    warm into the attention phase.
  - diag matrices are host-precomputed and DMA'd (frees ScalarE, removes the
    diag-build -> first-conv dependency).
  - input staging is split across 5 DMA rings (sync/scalar/vector/tensor/
    gpsimd) in conv-consumption order; xpad tiles get border-only memsets
    (the interior is fully overwritten by the DMA).
  - A=scale*rsqrt(var+eps) uses batched Ln-then-Exp (Ln and Exp live in
    different ACT table sets; interleaving them cost 11 x 1.6us table loads).

Layouts/folds (unchanged from previous version):
  - host passes inputs channel-major bf16 [b, c, h*w]; depthwise conv runs as
    9 diagonal-matmul taps, taps-outer so consecutive matmuls share one
    LDWEIGHTS per diag.
  - BN folds into the pointwise weights; bias via appended ones-row.
  - pre-softmax talking heads fold into K's weights; post-softmax talking
    heads AND w_out fold into V's weights (193-wide vw per head, col 192 = Z).
  - scores use fp8e4 DoubleRow (K=192 in one pass); sqrt(8)/8 on Q and
    1/sqrt(8) on K balances fp8 ranges.
  - qt is zero-padded to 3200 cols so every scores/AV tile is a full 128/512
    shape (no FD<128 DoubleRow penalty on the ragged 64-tail).
  - scores psum tiles are [112,3,512] (3 PSUM banks); one EXP per 3 taps
    (N=1536) instead of 7 per-bank EXPs -- ACT pays (N+352)/1.2ns per op, so
    fewer, larger EXPs cut ~50us of ScalarE overhead.
"""

import os
import sys
import functools

sys.path.insert(0, "/opt/trn_rl_repo")
os.environ.setdefault("MYCRO_LOCAL_CACHE", "1")

import numpy as np
import ml_dtypes

import concourse.bass as bass
import concourse.mybir as mybir
import concourse.tile as tile
from concourse import bacc
from concourse.bass_utils import run_bass_kernel_spmd

F32 = mybir.dt.float32
BF16 = mybir.dt.bfloat16
FP8 = mybir.dt.float8e4
DR = mybir.MatmulPerfMode.DoubleRow
QK_BAL = 0.35355339059327373   # sqrt(8)/8; applied to both Q and K folds
AF = mybir.ActivationFunctionType
ALU = mybir.AluOpType
AX = mybir.AxisListType

N_CORES = 8
BPC = 2                      # batches per core
C = 192                      # channels
HD = 192                     # num_heads * head_ch
NH = 3
LQ = 3136                    # 56*56
LQP = 3200                   # padded to 25*128 (tail cols are zeros)
LK = 784                     # 28*28
PADW = 58
PADN = PADW * PADW           # 3364
EPS = 1e-5

CT = [(0, 128), (128, 64)]   # channel tiles (partition dim)
KT112 = [(i * 112, 112) for i in range(7)]                # k_pos tiles
LCH = [(i * 512, 512) for i in range(6)] + [(3072, 128)]  # l chunks (padded)

last_results = None


def _emit(tc, nc, io, n_cores=N_CORES, mock_cc=False):
    (xq, xkv, diag_d, vecs_d, pwq_d, pwk_d, pwvT_d, wout_d, sm_d, out_d,
     cc_in, cc_out) = io
    MS = bass.MemorySpace

    with tc.tile_pool(name="wpool", bufs=1) as W, \
         tc.tile_pool(name="ypool", bufs=1) as Y:

        # ---------- static loads, spread over the 3 DMA rings ----------
        # Only SP (sync), Activation (scalar) and gpsimd can initiate DMAs,
        # and a DMA occupies its issuing engine's queue for the transfer.
        # Cross-engine deps are position-based (engine completion counters),
        # so each queue carries only what its consumers need, in consumption
        # order (convs run v, k, q):
        #   sync:   diag_v ci0 | xkv b0/b1 ci0 | diag_k | xq b0/b1 ci0 | weights
        #   scalar: diag_v ci1 | xkv b0/b1 ci1 | diag_q | xq b0/b1 ci1
        #   gpsimd: border memsets only, then the cc chain (AllReduce)
        diag = {}
        for p in range(3):
            for ci, (c0, csz) in enumerate(CT):
                diag[(p, ci)] = W.tile([csz, 9, csz], BF16, tag=f"diag{p}{ci}",
                                       name=f"diag{p}{ci}")

        # xpad tiles: all 4 (inp, b) pairs live concurrently
        xpad = {}
        for inp, b, ci in [(i, b, ci) for i in range(2) for b in range(BPC)
                           for ci in range(2)]:
            c0, csz = CT[ci]
            xp = W.tile([csz, PADN], BF16, tag=f"xp{inp}{b}{ci}", name=f"xp{inp}{b}{ci}")
            xpad[(inp, b, ci)] = xp

        def stage(inp, b, ci, eng):
            # host pre-pads to [C, 58*58], so staging is one contiguous DMA
            # (the old 8-row chunk DMAs moved 112-byte bursts at ~60 GB/s)
            c0, csz = CT[ci]
            src = xq if inp == 0 else xkv
            eng.dma_start(xpad[(inp, b, ci)][:, :], src.ap()[b, c0:c0 + csz, :])

        nc.sync.dma_start(diag[(2, 0)][:, :, :], diag_d[(2, 0)].ap())
        nc.scalar.dma_start(diag[(2, 1)][:, :, :], diag_d[(2, 1)].ap())
        stage(1, 0, 0, nc.sync)
        stage(1, 0, 1, nc.scalar)
        stage(1, 1, 0, nc.sync)
        stage(1, 1, 1, nc.scalar)
        nc.sync.dma_start(diag[(1, 0)][:, :, :], diag_d[(1, 0)].ap())
        nc.sync.dma_start(diag[(1, 1)][:, :, :], diag_d[(1, 1)].ap())
        nc.scalar.dma_start(diag[(0, 0)][:, :, :], diag_d[(0, 0)].ap())
        nc.scalar.dma_start(diag[(0, 1)][:, :, :], diag_d[(0, 1)].ap())
        stage(0, 0, 0, nc.sync)
        stage(0, 0, 1, nc.scalar)
        stage(0, 1, 0, nc.sync)
        stage(0, 1, 1, nc.scalar)

        # small weights (needed at fold time) on the sync ring, after staging
        vecs = []
        for ci, (c0, csz) in enumerate(CT):
            t = W.tile([csz, 6], F32, tag=f"vecs{ci}", name=f"vecs{ci}")
            nc.sync.dma_start(t[:, :], vecs_d.ap()[c0:c0 + csz, :])
            vecs.append(t)
        # smbc is host-prebuilt [128, 18] with QK_BAL folded into cols 0-8
        smbc = W.tile([128, 18], F32, tag="smbc")
        nc.sync.dma_start(smbc[:, :], sm_d.ap()[:, :])

        pwq_sb, pwk_sb, pwvT_sb, wout_sb = [], [], [], []
        for ci, (c0, csz) in enumerate(CT):
            for lst, dram, nm in ((pwq_sb, pwq_d, "pwq"), (pwk_sb, pwk_d, "pwk"),
                                  (pwvT_sb, pwvT_d, "pwvT"), (wout_sb, wout_d, "wout")):
                t = W.tile([csz, 192], F32, tag=f"{nm}{ci}", name=f"{nm}{ci}")
                nc.sync.dma_start(t[:, :], dram.ap()[c0:c0 + csz, :])
                lst.append(t)

        # conv outputs (augmented with ones row on tile 2)
        ysz = {0: LQ, 1: LK, 2: LK}
        y = {}
        for b in range(BPC):
            for p in range(3):
                y[(b, p, 0)] = Y.tile([128, ysz[p]], BF16, tag=f"y{b}{p}0", name=f"y{b}{p}0")
                y[(b, p, 1)] = Y.tile([65, ysz[p]], BF16, tag=f"y{b}{p}1", name=f"y{b}{p}1")
                nc.vector.memset(y[(b, p, 1)][64:65, :], 1.0)

        # per-path bn_stats slots: q 14 groups, k/v 4 groups of 6
        slots = {}
        for p, ngrp in ((0, 14), (1, 4), (2, 4)):
            slots[p] = [W.tile([csz, 6 * ngrp], F32, tag=f"sl{p}{ci}", name=f"sl{p}{ci}")
                        for ci, (c0, csz) in enumerate(CT)]
        mv = {p: [W.tile([csz, 2], F32, tag=f"mv{p}{ci}", name=f"mv{p}{ci}")
                  for ci, (c0, csz) in enumerate(CT)] for p in range(3)}
        ccst = [W.tile([csz, 3], F32, tag=f"ccst{ci}", name=f"ccst{ci}")
                for ci, (c0, csz) in enumerate(CT)]
        gst = [W.tile([csz, 2], F32, tag=f"gst{ci}", name=f"gst{ci}")
               for ci, (c0, csz) in enumerate(CT)]

        # phase-2 tiles
        # ab cols: [A_q' 0 | A_k 1 | A_v 2 | mean_q 3 | mean_k 4 | mean_v 5]
        ab = [W.tile([csz, 6], F32, tag=f"ab{ci}", name=f"ab{ci}")
              for ci, (c0, csz) in enumerate(CT)]
        bbf = [W.tile([csz, 3], BF16, tag=f"bbf{ci}", name=f"bbf{ci}")
               for ci, (c0, csz) in enumerate(CT)]
        vep = [W.tile([csz, 3], F32, tag=f"vep{ci}", name=f"vep{ci}")
               for ci, (c0, csz) in enumerate(CT)]
        lt = [W.tile([csz, 3], F32, tag=f"lt{ci}", name=f"lt{ci}")
              for ci, (c0, csz) in enumerate(CT)]
        rstd = [W.tile([csz, 3], F32, tag=f"rstd{ci}", name=f"rstd{ci}")
                for ci, (c0, csz) in enumerate(CT)]
        tmp = [W.tile([csz, 2], F32, tag=f"tmp{ci}", name=f"tmp{ci}")
               for ci, (c0, csz) in enumerate(CT)]
        NTOT_V = float(n_cores * BPC * LK)

        pwqA = [W.tile([128, 192], BF16, tag="pwqA0", name="pwqA0"),
                W.tile([65, 192], BF16, tag="pwqA1", name="pwqA1")]
        pwkA = [W.tile([csz, 192], BF16, tag=f"pwkA{ci}", name=f"pwkA{ci}")
                for ci, (c0, csz) in enumerate(CT)]
        browk = W.tile([1, 192], F32, tag="browk")
        pwvT_bf = [W.tile([csz, 192], BF16, tag=f"pwvTb{ci}", name=f"pwvTb{ci}")
                   for ci, (c0, csz) in enumerate(CT)]
        postvec = W.tile([128, 3], F32, tag="postvec")
        wbar = [W.tile([128, 192], BF16, tag="wbar0", name="wbar0"),
                W.tile([64, 192], BF16, tag="wbar1", name="wbar1")]
        kw, cw = {}, {}
        for g in range(NH):
            kw[(g, 0)] = W.tile([128, 192], BF16, tag=f"kw{g}0", name=f"kw{g}0")
            kw[(g, 1)] = W.tile([65, 192], BF16, tag=f"kw{g}1", name=f"kw{g}1")
            cw[(g, 0)] = W.tile([128, 193], BF16, tag=f"cw{g}0", name=f"cw{g}0")
            cw[(g, 1)] = W.tile([65, 193], BF16, tag=f"cw{g}1", name=f"cw{g}1")

        def ab_chain(paths):
            # A = scale * rsqrt(var+eps), b'' = offset/A - mean; batched Ln
            # pass then batched Exp pass (Ln and Exp are in different ACT
            # table sets -- interleaving would reload tables per op)
            p0, p1 = min(paths), max(paths) + 1
            for ci, (c0, csz) in enumerate(CT):
                for p in paths:
                    if p < 2:  # q, k: local batch stats
                        nc.vector.tensor_scalar(ab[ci][:, 3 + p:4 + p],
                                                mv[p][ci][:, 0:1], 1.0, None, ALU.mult)
                        nc.vector.tensor_scalar(vep[ci][:, p:p + 1], mv[p][ci][:, 1:2],
                                                1.0, EPS, ALU.mult, ALU.add)
                    else:      # v: global stats from the AllReduce
                        inv_n = 1.0 / NTOT_V
                        mean_v = ab[ci][:, 5:6]
                        nc.vector.tensor_scalar(mean_v, gst[ci][:, 0:1], inv_n,
                                                None, ALU.mult)
                        nc.vector.tensor_scalar(tmp[ci][:, 0:1], gst[ci][:, 1:2],
                                                inv_n, EPS, ALU.mult, ALU.add)
                        nc.vector.tensor_scalar(tmp[ci][:, 1:2], mean_v, mean_v,
                                                None, ALU.mult)
                        nc.vector.tensor_tensor(vep[ci][:, 2:3], tmp[ci][:, 0:1],
                                                tmp[ci][:, 1:2], ALU.subtract)
                nc.scalar.activation(lt[ci][:, p0:p1], vep[ci][:, p0:p1], AF.Ln)
            for ci, (c0, csz) in enumerate(CT):
                nc.scalar.activation(rstd[ci][:, p0:p1], lt[ci][:, p0:p1],
                                     AF.Exp, scale=-0.5)
                for p in paths:
                    A = ab[ci][:, p:p + 1]
                    nc.vector.tensor_scalar(A, rstd[ci][:, p:p + 1],
                                            vecs[ci][:, 2 * p:2 * p + 1], None, ALU.mult)
                    recA = tmp[ci][:, 0:1]
                    nc.vector.reciprocal(recA, A)
                    bpp = tmp[ci][:, 1:2]       # b'' = offset*recA - mean
                    nc.vector.scalar_tensor_tensor(bpp, vecs[ci][:, 2 * p + 1:2 * p + 2],
                                                   recA, ab[ci][:, 3 + p:4 + p],
                                                   ALU.mult, ALU.subtract)
                    nc.vector.tensor_scalar(bbf[ci][:, p:p + 1], bpp, 1.0, None, ALU.mult)
                    if p == 0:
                        nc.vector.tensor_scalar(A, A, QK_BAL, None, ALU.mult)

        # ---------- phase 1+2: convs (order v, k, q) with folds interleaved ----------
        with tc.tile_pool(name="pconv", bufs=6, space=MS.PSUM) as PCONV, \
             tc.tile_pool(name="prow", bufs=1, space=MS.PSUM) as PROW, \
             tc.tile_pool(name="pcw", bufs=1, space=MS.PSUM) as PCW:

            def conv_kv(b, p):
                # stride 2 over xkv, psum chunks of 392 (14 output rows);
                # psum->y copies on VectorE (ScalarE is busy staging xq)
                for ci, (c0, csz) in enumerate(CT):
                    xv = xpad[(1, b, ci)].rearrange("p (h th w tw) -> p h th w tw",
                                                    th=2, tw=2, w=29)
                    pss = [PCONV.tile([csz, 392], F32, tag="convps", name="convps")
                           for _ in range(2)]
                    t = 0
                    for dy in (0, 1, 2):
                        for dx in (0, 1, 2):
                            for kc in range(2):
                                h0, th = divmod(28 * kc + dy + 1, 2)
                                w0, tw = divmod(dx + 1, 2)
                                rhs = xv[0:csz, h0:h0 + 14, th, w0:w0 + 28, tw]
                                nc.tensor.matmul(pss[kc][:, :], diag[(p, ci)][:, t, :], rhs,
                                                 start=(t == 0), stop=(t == 8))
                            t += 1
                    for kc in range(2):
                        si = 2 * b + kc
                        ysl = y[(b, p, ci)][0:csz, 392 * kc:392 * (kc + 1)]
                        nc.vector.tensor_copy(ysl, pss[kc][:, :])
                        nc.vector.bn_stats(slots[p][ci][:, 6 * si:6 * si + 6], ysl)

            def conv_q(b):
                # stride 1, psum chunks of 448 (8 output rows); taps outer so
                # consecutive matmuls share one diag LDWEIGHTS. Chunks run in
                # two groups of 4+3 so the psum->y copies free PCONV tiles at
                # the halfway point -- with all 7 chunks completing only on the
                # last tap, the next batch's convs stalled ~2.3us on psum
                # recycling, long enough for HAM to re-throttle the PE.
                for ci, (c0, csz) in enumerate(CT):
                    xv = xpad[(0, b, ci)].rearrange("p (h w) -> p h w", w=PADW)
                    for qc0, nqc in ((0, 4), (4, 3)):
                        pss = [PCONV.tile([csz, 448], F32, tag="convps", name="convps")
                               for _ in range(nqc)]
                        t = 0
                        for dy in (-1, 0, 1):
                            for dx in (-1, 0, 1):
                                for j in range(nqc):
                                    r0 = 8 * (qc0 + j) + 1 + dy
                                    rhs = xv[0:csz, r0:r0 + 8, 1 + dx:57 + dx]
                                    nc.tensor.matmul(pss[j][:, :], diag[(0, ci)][:, t, :], rhs,
                                                     start=(t == 0), stop=(t == 8))
                                t += 1
                        for j in range(nqc):
                            si = 7 * b + qc0 + j
                            ysl = y[(b, 0, ci)][0:csz, 448 * (qc0 + j):448 * (qc0 + j + 1)]
                            nc.scalar.activation(ysl, pss[j][:, :], AF.Copy)
                            nc.vector.bn_stats(slots[0][ci][:, 6 * si:6 * si + 6], ysl)

            conv_kv(0, 2)
            conv_kv(1, 2)

            # v stats -> (sum, sumsq) -> AllReduce, launched under the k/q convs
            NLOC_V = float(BPC * LK)
            for ci, (c0, csz) in enumerate(CT):
                nc.vector.bn_aggr(mv[2][ci][:, 0:2], slots[2][ci][:, 0:24])
                m = mv[2][ci][:, 0:1]
                v = mv[2][ci][:, 1:2]
                nc.vector.tensor_scalar(ccst[ci][:, 0:1], m, NLOC_V, None, ALU.mult)
                nc.vector.tensor_scalar(ccst[ci][:, 2:3], m, m, None, ALU.mult)
                nc.vector.tensor_scalar(ccst[ci][:, 1:2], v, ccst[ci][:, 2:3],
                                        NLOC_V, ALU.add, ALU.mult)
                nc.gpsimd.dma_start(cc_in.ap()[c0:c0 + csz, :], ccst[ci][:, 0:2])
            if mock_cc:
                nc.gpsimd.dma_start(cc_out.ap()[:, :], cc_in.ap()[:, :])
            else:
                nc.gpsimd.collective_compute(
                    "AllReduce", ALU.add, replica_groups=[list(range(n_cores))],
                    ins=[cc_in.ap()], outs=[cc_out.ap()])
            for ci, (c0, csz) in enumerate(CT):
                nc.gpsimd.dma_start(gst[ci][:, :], cc_out.ap()[c0:c0 + csz, :])

            conv_kv(0, 1)
            conv_kv(1, 1)
            for ci in range(2):
                nc.vector.bn_aggr(mv[1][ci][:, 0:2], slots[1][ci][:, 0:24])

            conv_q(0)

            # ----- fold k while the PE runs q convs (local stats only, so
            # the PE's kw bias-row matmul can't wedge on the AllReduce) -----
            ab_chain((1,))

            # k weights + pre_sm folding
            for ci, (c0, csz) in enumerate(CT):
                nc.vector.tensor_scalar(pwkA[ci][:, :], pwk_sb[ci][:, :],
                                        ab[ci][:, 1:2], None, ALU.mult)
            rps = PROW.tile([1, 192], F32, tag="rowps", name="rowps")
            nc.tensor.matmul(rps[:, :], bbf[0][:, 1:2], pwkA[0][:, :], start=True, stop=False)
            nc.tensor.matmul(rps[:, :], bbf[1][:, 1:2], pwkA[1][:, :], start=False, stop=True)
            nc.vector.tensor_copy(browk[:, :], rps[:, :])
            for g in range(NH):
                for h in range(NH):
                    col = 3 * h + g
                    for ci, (c0, csz) in enumerate(CT):
                        nc.vector.tensor_scalar(kw[(g, ci)][0:csz, 64 * h:64 * (h + 1)],
                                                pwkA[ci][:, 64 * h:64 * (h + 1)],
                                                smbc[0:csz, col:col + 1], None, ALU.mult)
                    nc.vector.tensor_scalar(kw[(g, 1)][64:65, 64 * h:64 * (h + 1)],
                                            browk[:, 64 * h:64 * (h + 1)],
                                            smbc[0:1, col:col + 1], None, ALU.mult)

            conv_q(1)

            # ----- fold v (needs the AllReduce, long since landed) -----
            ab_chain((2,))

            # v weights: cw_aug_g with post_sm + w_out folded
            for ci, (c0, csz) in enumerate(CT):
                nc.vector.tensor_copy(pwvT_bf[ci][:, :], pwvT_sb[ci][:, :])
            for g in range(NH):
                nc.vector.tensor_copy(postvec[0:64, g:g + 1], smbc[0:64, 9 + 3 * g:10 + 3 * g])
                nc.vector.tensor_copy(postvec[64:128, g:g + 1], smbc[64:128, 10 + 3 * g:11 + 3 * g])
                nc.vector.tensor_scalar(wbar[0][:, :], wout_sb[0][:, :],
                                        postvec[:, g:g + 1], None, ALU.mult)
                nc.vector.tensor_scalar(wbar[1][:, :], wout_sb[1][:, :],
                                        smbc[0:64, 11 + 3 * g:12 + 3 * g], None, ALU.mult)
                for ci, (c0, csz) in enumerate(CT):
                    ps = PCW.tile([csz, 192], F32, tag="cwps", name="cwps")
                    nc.tensor.matmul(ps[:, :], pwvT_bf[0][:, c0:c0 + csz], wbar[0][:, :],
                                     start=True, stop=False)
                    nc.tensor.matmul(ps[:, :], pwvT_bf[1][:, c0:c0 + csz], wbar[1][:, :],
                                     start=False, stop=True)
                    nc.scalar.activation(cw[(g, ci)][0:csz, 0:192], ps[:, :], AF.Copy,
                                         scale=ab[ci][:, 2:3])
                rps = PROW.tile([1, 192], F32, tag="rowps", name="rowps")
                nc.tensor.matmul(rps[:, :], bbf[0][:, 2:3], cw[(g, 0)][0:128, 0:192],
                                 start=True, stop=False)
                nc.tensor.matmul(rps[:, :], bbf[1][:, 2:3], cw[(g, 1)][0:64, 0:192],
                                 start=False, stop=True)
                nc.vector.tensor_copy(cw[(g, 1)][64:65, 0:192], rps[:, :])
                nc.gpsimd.memset(cw[(g, 0)][:, 192:193], 0.0)
                nc.gpsimd.memset(cw[(g, 1)][0:64, 192:193], 0.0)
                nc.gpsimd.memset(cw[(g, 1)][64:65, 192:193], 1.0)

            for ci in range(2):
                nc.vector.bn_aggr(mv[0][ci][:, 0:2], slots[0][ci][:, 0:84])

        # ---------- phase 3: projections + attention ----------
        with tc.tile_pool(name="qtpool", bufs=2) as QT, \
             tc.tile_pool(name="ktpool", bufs=2) as KTP, \
             tc.tile_pool(name="vwpool", bufs=2) as VWP, \
             tc.tile_pool(name="epool", bufs=2) as EP, \
             tc.tile_pool(name="accpool", bufs=3) as ACC, \
             tc.tile_pool(name="outpool", bufs=3) as OUT, \
             tc.tile_pool(name="rzpool", bufs=8) as RZ, \
             tc.tile_pool(name="stri", bufs=2, space=MS.PSUM) as STRI, \
             tc.tile_pool(name="pav", bufs=2, space=MS.PSUM) as PAV:

            for b in range(BPC):
                # q/k projections land in fp8 [96, 2, L]: channel c = 96*i + p,
                # the layout the DoubleRow score matmul contracts directly.
                kt = {}
                for g in range(NH):
                    kt[g] = KTP.tile([96, 2, LK], FP8, tag=f"kt{g}", name=f"kt{g}")
                    for k0 in (0, 392):
                        for mi in range(2):
                            ps3 = STRI.tile([112, 3, 512], F32, tag="stri", name="stri")
                            ps = ps3[0:96, 0, 0:392]
                            nc.tensor.matmul(ps, kw[(g, 0)][:, 96 * mi:96 * mi + 96],
                                             y[(b, 1, 0)][:, k0:k0 + 392], start=True, stop=False)
                            nc.tensor.matmul(ps, kw[(g, 1)][:, 96 * mi:96 * mi + 96],
                                             y[(b, 1, 1)][:, k0:k0 + 392], start=False, stop=True)
                            nc.vector.tensor_copy(kt[g][:, mi, k0:k0 + 392], ps)

                if b == 0:
                    # ----- fold q: its vector/scalar chain runs under the kt
                    # projection above (which only needs kw) -----
                    ab_chain((0,))
                    for ci, (c0, csz) in enumerate(CT):
                        nc.vector.tensor_scalar(pwqA[ci][0:csz, :], pwq_sb[ci][:, :],
                                                ab[ci][:, 0:1], None, ALU.mult)
                    rps = STRI.tile([112, 3, 512], F32, tag="stri",
                                    name="stri")[0:1, 0, 0:192]
                    nc.tensor.matmul(rps, bbf[0][:, 0:1], pwqA[0][0:128, :],
                                     start=True, stop=False)
                    nc.tensor.matmul(rps, bbf[1][:, 0:1], pwqA[1][0:64, :],
                                     start=False, stop=True)
                    nc.vector.tensor_copy(pwqA[1][64:65, :], rps)

                vw = {}
                for g in range(NH):
                    vw[g] = VWP.tile([112, 7, 193], BF16, tag=f"vw{g}", name=f"vw{g}")
                    for ki, (k0, ksz) in enumerate(KT112):
                        ps = PAV.tile([128, 193], F32, tag="avps", name="avps")[0:112, :]
                        nc.tensor.matmul(ps, y[(b, 2, 0)][:, k0:k0 + ksz], cw[(g, 0)][:, :],
                                         start=True, stop=False)
                        nc.tensor.matmul(ps, y[(b, 2, 1)][:, k0:k0 + ksz], cw[(g, 1)][:, :],
                                         start=False, stop=True)
                        nc.vector.tensor_copy(vw[g][:, ki, :], ps)

                qt = QT.tile([96, 2, LQP], FP8, tag="qt", name="qt")
                for mi in range(2):
                    nc.gpsimd.memset(qt[:, mi, LQ:LQP], 0.0)
                for l0, lsz in [(i * 512, 512) for i in range(6)] + [(3072, 64)]:
                    for mi in range(2):
                        ps3 = STRI.tile([112, 3, 512], F32, tag="stri", name="stri")
                        ps = ps3[0:96, 0, 0:lsz]
                        nc.tensor.matmul(ps, pwqA[0][:, 96 * mi:96 * mi + 96],
                                         y[(b, 0, 0)][:, l0:l0 + lsz], start=True, stop=False)
                        nc.tensor.matmul(ps, pwqA[1][:, 96 * mi:96 * mi + 96],
                                         y[(b, 0, 1)][:, l0:l0 + lsz], start=False, stop=True)
                        nc.vector.tensor_copy(qt[:, mi, l0:l0 + lsz], ps)

                for l0, lsz in LCH:
                    ee = {}
                    for g in range(NH):
                        ee[g] = EP.tile([112, 7, 512], BF16, tag=f"e{g}", name=f"e{g}")
                        for g0 in (0, 3, 6):
                            ng = min(3, 7 - g0)
                            ps3 = STRI.tile([112, 3, 512], F32, tag="stri", name="stri")
                            for j in range(ng):
                                k0, ksz = KT112[g0 + j]
                                nc.tensor.matmul(ps3[:, j, 0:lsz], kt[g][:, :, k0:k0 + ksz],
                                                 qt[:, :, l0:l0 + lsz],
                                                 start=True, stop=True, perf_mode=DR)
                            nc.scalar.activation(ee[g][:, g0:g0 + ng, 0:lsz],
                                                 ps3[:, 0:ng, 0:lsz], AF.Exp)

                    for ls in range(0, lsz, 128):
                        acc = ACC.tile([128, 192], F32, tag="acc", name="acc")
                        ot = OUT.tile([128, 192], F32, tag="ot", name="ot")
                        for g in range(NH):
                            ps = PAV.tile([128, 193], F32, tag="avps", name="avps")
                            pv = ps[0:128, 0:193]
                            for ki in range(7):
                                nc.tensor.matmul(pv, ee[g][:, ki, ls:ls + 128],
                                                 vw[g][:, ki, :],
                                                 start=(ki == 0), stop=(ki == 6))
                            r = RZ.tile([128, 1], F32, tag="rz", name="rz")
                            nc.vector.reciprocal(r[:, :], pv[:, 192:193])
                            if g == 0:
                                nc.vector.tensor_scalar(acc[:, :], pv[:, 0:192], r[:, :],
                                                        None, ALU.mult)
                            elif g == 1:
                                nc.vector.scalar_tensor_tensor(acc[:, :], pv[:, 0:192],
                                                               r[:, :], acc[:, :],
                                                               ALU.mult, ALU.add)
                            else:
                                nc.vector.scalar_tensor_tensor(ot[:, :], pv[:, 0:192],
                                                               r[:, :], acc[:, :],
                                                               ALU.mult, ALU.add)
                        lw_out = 64 if l0 + ls == 3072 else 128
                        nc.sync.dma_start(out_d.ap()[b, l0 + ls:l0 + ls + lw_out, :],
                                          ot[0:lw_out, :])


def build(n_cores=N_CORES, mock_cc=False):
    nc = bacc.Bacc("TRN2", target_bir_lowering=False, debug=False, num_devices=n_cores)
    xq = nc.dram_tensor("xq", [BPC, C, PADN], BF16, kind="ExternalInput")
    xkv = nc.dram_tensor("xkv", [BPC, C, PADN], BF16, kind="ExternalInput")
    diag_d = {}
    for p in range(3):
        for ci, (c0, csz) in enumerate(CT):
            diag_d[(p, ci)] = nc.dram_tensor(f"diag{p}{ci}", [csz, 9, csz], BF16,
                                             kind="ExternalInput")
    vecs_d = nc.dram_tensor("vecs", [C, 6], F32, kind="ExternalInput")
    pwq_d = nc.dram_tensor("pwq", [C, HD], F32, kind="ExternalInput")
    pwk_d = nc.dram_tensor("pwk", [C, HD], F32, kind="ExternalInput")
    pwvT_d = nc.dram_tensor("pwvT", [HD, C], F32, kind="ExternalInput")
    wout_d = nc.dram_tensor("wout", [HD, C], F32, kind="ExternalInput")
    sm_d = nc.dram_tensor("smm", [128, 18], F32, kind="ExternalInput")
    out_d = nc.dram_tensor("out", [BPC, LQ, C], F32, kind="ExternalOutput")
    cc_in = nc.dram_tensor("cc_in", [C, 2], F32)
    cc_out = nc.dram_tensor("cc_out", [C, 2], F32)

    io = (xq, xkv, diag_d, vecs_d, pwq_d, pwk_d, pwvT_d, wout_d, sm_d, out_d,
          cc_in, cc_out)
    with tile.TileContext(nc) as tc:
        _emit(tc, nc, io, n_cores=n_cores, mock_cc=mock_cc)
    nc.compile()
    return nc


@functools.lru_cache(maxsize=1)
def _built():
    return build()


def make_in_maps(inputs):
    f32c = lambda a: np.ascontiguousarray(np.asarray(a), dtype=np.float32)
    bf16c = lambda a: np.ascontiguousarray(np.asarray(a, dtype=np.float32)
                                           .astype(ml_dtypes.bfloat16))
    def pad_cm(a):  # [16,56,56,C] -> channel-major zero-padded [16, C, 58*58]
        t = np.asarray(a, dtype=np.float32).transpose(0, 3, 1, 2)
        p = np.zeros((16, C, PADW, PADW), dtype=ml_dtypes.bfloat16)
        p[:, :, 1:57, 1:57] = t.astype(ml_dtypes.bfloat16)
        return np.ascontiguousarray(p.reshape(16, C, PADN))

    xq_t = pad_cm(inputs["inputs_q"])
    xkv_t = pad_cm(inputs["inputs_kv"])
    dwt = np.concatenate([np.asarray(inputs[k], dtype=np.float32).reshape(9, C).T
                          for k in ("dw_q", "dw_k", "dw_v")], axis=1)  # [C, 27]
    base = {
        "vecs": f32c(np.stack([np.asarray(inputs[k]) for k in
                               ("scale_q", "offset_q", "scale_k", "offset_k",
                                "scale_v", "offset_v")], axis=1)),
        "smm": f32c(np.broadcast_to(np.concatenate(
            [np.asarray(inputs["pre_sm"]).reshape(-1) * QK_BAL,
             np.asarray(inputs["post_sm"]).reshape(-1)])[None, :], (128, 18))),
        "pwq": f32c(inputs["pw_q"]), "pwk": f32c(inputs["pw_k"]),
        "pwvT": f32c(np.asarray(inputs["pw_v"]).T), "wout": f32c(inputs["w_out"]),
    }
    for p in range(3):
        for ci, (c0, csz) in enumerate(CT):
            d = np.zeros((csz, 9, csz), dtype=ml_dtypes.bfloat16)
            blk = dwt[c0:c0 + csz, 9 * p:9 * p + 9].astype(ml_dtypes.bfloat16)
            d[np.arange(csz)[:, None], np.arange(9)[None, :], np.arange(csz)[:, None]] = blk
            base[f"diag{p}{ci}"] = d
    in_maps = []
    for i in range(N_CORES):
        m = dict(base)
        m["xq"] = np.ascontiguousarray(xq_t[BPC * i:BPC * (i + 1)])
        m["xkv"] = np.ascontiguousarray(xkv_t[BPC * i:BPC * (i + 1)])
        in_maps.append(m)
    return in_maps


def kernel(**inputs):
    global last_results
    nc = _built()
    in_maps = make_in_maps(inputs)
    trace = os.environ.get("BASS_KERNEL_TRACE", "0") == "1"
    res = run_bass_kernel_spmd(nc, in_maps, core_ids=list(range(N_CORES)), trace=trace)
    last_results = res
    out = np.concatenate([res.results[i]["out"] for i in range(N_CORES)], axis=0)
    return out.astype(np.float32)


if __name__ == "__main__":
    import reference
    inputs = reference.setup_inputs()
    expected = np.asarray(reference.reference(**inputs))
    actual = kernel(**inputs)
    d = np.abs(actual - expected)
    print(f"absmax={d.max():.3e} scale={np.abs(expected).max():.3e} "
          f"rel={d.max() / np.abs(expected).max():.3e}")


# revision 37
# speedup vs baseline: 1.2401x; 1.1592x over previous
"""CvT attention block (depthwise conv proj + BN + talking-heads attention) on 8 trn2 cores.

Sharding: data-parallel over batch (16 batches -> 2 per core).

BN stats: only V needs the cross-core AllReduce. A per-core K mean error is
softmax-shift-invariant (constant across k for each (g,l), even through the
pre_sm head mix); a Q mean error perturbs logits by eps.k_j, incoherent over k;
q/k var errors from 6272/1568 local samples are ~1-2% scale noise that washes
out. V's mean error is channel-coherent and survives the 1/784 attention
average (measured 0.54 rel err with local V stats), so V stats are AllReduced.

Schedule (the previous version stalled ~40us at a late 6x192 AllReduce and then
ran ~70us at K=4/8 because HAM re-throttled the idle PE):
  - conv order v, k, q; the V-stats AllReduce (192x2) launches after the v
    convs (~25us in) and lands during the q convs -- zero PE stall, PE stays
    warm into the attention phase.
  - diag matrices are host-precomputed and DMA'd (frees ScalarE, removes the
    diag-build -> first-conv dependency).
  - input staging is split across 5 DMA rings (sync/scalar/vector/tensor/
    gpsimd) in conv-consumption order; xpad tiles get border-only memsets
    (the interior is fully overwritten by the DMA).
  - A=scale*rsqrt(var+eps) uses batched Ln-then-Exp (Ln and Exp live in
    different ACT table sets; interleaving them cost 11 x 1.6us table loads).

Layouts/folds (unchanged from previous version):
  - host passes inputs channel-major bf16 [b, c, h*w]; depthwise conv runs as
    9 diagonal-matmul taps, taps-outer so consecutive matmuls share one
    LDWEIGHTS per diag.
  - BN folds into the pointwise weights; bias via appended ones-row.
  - pre-softmax talking heads fold into K's weights; post-softmax talking
    heads AND w_out fold into V's weights (193-wide vw per head, col 192 = Z).
  - scores use fp8e4 DoubleRow (K=192 in one pass); sqrt(8)/8 on Q and
    1/sqrt(8) on K balances fp8 ranges.
  - qt is zero-padded to 3200 cols so every scores/AV tile is a full 128/512
    shape (no FD<128 DoubleRow penalty on the ragged 64-tail).
  - scores psum tiles are [112,3,512] (3 PSUM banks); one EXP per 3 taps
    (N=1536) instead of 7 per-bank EXPs -- ACT pays (N+352)/1.2ns per op, so
    fewer, larger EXPs cut ~50us of ScalarE overhead.
"""

import os
import sys
import functools

sys.path.insert(0, "/opt/trn_rl_repo")
os.environ.setdefault("MYCRO_LOCAL_CACHE", "1")

import numpy as np
import ml_dtypes

import concourse.bass as bass
import concourse.mybir as mybir
import concourse.tile as tile
from concourse import bacc
from concourse.bass_utils import run_bass_kernel_spmd

F32 = mybir.dt.float32
BF16 = mybir.dt.bfloat16
FP8 = mybir.dt.float8e4
DR = mybir.MatmulPerfMode.DoubleRow
QK_BAL = 0.35355339059327373   # sqrt(8)/8; applied to both Q and K folds
AF = mybir.ActivationFunctionType
ALU = mybir.AluOpType
AX = mybir.AxisListType

N_CORES = 8
BPC = 2                      # batches per core
C = 192                      # channels
HD = 192                     # num_heads * head_ch
NH = 3
LQ = 3136                    # 56*56
LQP = 3200                   # padded to 25*128 (tail cols are zeros)
LK = 784                     # 28*28
PADW = 58
PADN = PADW * PADW           # 3364
EPS = 1e-5

CT = [(0, 128), (128, 64)]   # channel tiles (partition dim)
KT112 = [(i * 112, 112) for i in range(7)]                # k_pos tiles
LCH = [(i * 512, 512) for i in range(6)] + [(3072, 128)]  # l chunks (padded)

last_results = None


def _emit(tc, nc, io, n_cores=N_CORES, mock_cc=False):
    (xq, xkv, diag_d, vecs_d, pwq_d, pwk_d, pwvT_d, wout_d, sm_d, out_d,
     cc_in, cc_out) = io
    MS = bass.MemorySpace

    with tc.tile_pool(name="wpool", bufs=1) as W, \
         tc.tile_pool(name="ypool", bufs=1) as Y:

        # ---------- static loads, spread over the 3 DMA rings ----------
        # Only SP (sync), Activation (scalar) and gpsimd can initiate DMAs,
        # and a DMA occupies its issuing engine's queue for the transfer.
        # Cross-engine deps are position-based (engine completion counters),
        # so each queue carries only what its consumers need, in consumption
        # order (convs run v, k, q):
        #   sync:   diag_v ci0 | xkv b0/b1 ci0 | diag_k | xq b0/b1 ci0 | weights
        #   scalar: diag_v ci1 | xkv b0/b1 ci1 | diag_q | xq b0/b1 ci1
        #   gpsimd: border memsets only, then the cc chain (AllReduce)
        diag = {}
        for p in range(3):
            for ci, (c0, csz) in enumerate(CT):
                diag[(p, ci)] = W.tile([csz, 9, csz], BF16, tag=f"diag{p}{ci}",
                                       name=f"diag{p}{ci}")

        # xpad tiles: all 4 (inp, b) pairs live concurrently
        xpad = {}
        for inp, b, ci in [(i, b, ci) for i in range(2) for b in range(BPC)
                           for ci in range(2)]:
            c0, csz = CT[ci]
            xp = W.tile([csz, PADN], BF16, tag=f"xp{inp}{b}{ci}", name=f"xp{inp}{b}{ci}")
            xpad[(inp, b, ci)] = xp

        def stage(inp, b, ci, eng):
            # host pre-pads to [C, 58*58], so staging is one contiguous DMA
            # (the old 8-row chunk DMAs moved 112-byte bursts at ~60 GB/s)
            c0, csz = CT[ci]
            src = xq if inp == 0 else xkv
            eng.dma_start(xpad[(inp, b, ci)][:, :], src.ap()[b, c0:c0 + csz, :])

        nc.sync.dma_start(diag[(2, 0)][:, :, :], diag_d[(2, 0)].ap())
        nc.scalar.dma_start(diag[(2, 1)][:, :, :], diag_d[(2, 1)].ap())
        stage(1, 0, 0, nc.sync)
        stage(1, 0, 1, nc.scalar)
        stage(1, 1, 0, nc.sync)
        stage(1, 1, 1, nc.scalar)
        nc.sync.dma_start(diag[(1, 0)][:, :, :], diag_d[(1, 0)].ap())
        nc.sync.dma_start(diag[(1, 1)][:, :, :], diag_d[(1, 1)].ap())
        nc.scalar.dma_start(diag[(0, 0)][:, :, :], diag_d[(0, 0)].ap())
        nc.scalar.dma_start(diag[(0, 1)][:, :, :], diag_d[(0, 1)].ap())
        stage(0, 0, 0, nc.sync)
        stage(0, 0, 1, nc.scalar)
        stage(0, 1, 0, nc.sync)
        stage(0, 1, 1, nc.scalar)

        # small weights (needed at fold time) on the sync ring, after staging
        vecs = []
        for ci, (c0, csz) in enumerate(CT):
            t = W.tile([csz, 6], F32, tag=f"vecs{ci}", name=f"vecs{ci}")
            nc.sync.dma_start(t[:, :], vecs_d.ap()[c0:c0 + csz, :])
            vecs.append(t)
        # smbc is host-prebuilt [128, 18] with QK_BAL folded into cols 0-8
        smbc = W.tile([128, 18], F32, tag="smbc")
        nc.sync.dma_start(smbc[:, :], sm_d.ap()[:, :])

        pwq_sb, pwk_sb, pwvT_sb, wout_sb = [], [], [], []
        for ci, (c0, csz) in enumerate(CT):
            for lst, dram, nm in ((pwq_sb, pwq_d, "pwq"), (pwk_sb, pwk_d, "pwk"),
                                  (pwvT_sb, pwvT_d, "pwvT"), (wout_sb, wout_d, "wout")):
                t = W.tile([csz, 192], F32, tag=f"{nm}{ci}", name=f"{nm}{ci}")
                nc.sync.dma_start(t[:, :], dram.ap()[c0:c0 + csz, :])
                lst.append(t)

        # conv outputs (augmented with ones row on tile 2)
        ysz = {0: LQ, 1: LK, 2: LK}
        y = {}
        for b in range(BPC):
            for p in range(3):
                y[(b, p, 0)] = Y.tile([128, ysz[p]], BF16, tag=f"y{b}{p}0", name=f"y{b}{p}0")
                y[(b, p, 1)] = Y.tile([65, ysz[p]], BF16, tag=f"y{b}{p}1", name=f"y{b}{p}1")
                nc.vector.memset(y[(b, p, 1)][64:65, :], 1.0)

        # per-path bn_stats slots: q 14 groups, k/v 4 groups of 6
        slots = {}
        for p, ngrp in ((0, 14), (1, 4), (2, 4)):
            slots[p] = [W.tile([csz, 6 * ngrp], F32, tag=f"sl{p}{ci}", name=f"sl{p}{ci}")
                        for ci, (c0, csz) in enumerate(CT)]
        mv = {p: [W.tile([csz, 2], F32, tag=f"mv{p}{ci}", name=f"mv{p}{ci}")
                  for ci, (c0, csz) in enumerate(CT)] for p in range(3)}
        ccst = [W.tile([csz, 3], F32, tag=f"ccst{ci}", name=f"ccst{ci}")
                for ci, (c0, csz) in enumerate(CT)]
        gst = [W.tile([csz, 2], F32, tag=f"gst{ci}", name=f"gst{ci}")
               for ci, (c0, csz) in enumerate(CT)]

        # phase-2 tiles
        # ab cols: [A_q' 0 | A_k 1 | A_v 2 | mean_q 3 | mean_k 4 | mean_v 5]
        ab = [W.tile([csz, 6], F32, tag=f"ab{ci}", name=f"ab{ci}")
              for ci, (c0, csz) in enumerate(CT)]
        bbf = [W.tile([csz, 3], BF16, tag=f"bbf{ci}", name=f"bbf{ci}")
               for ci, (c0, csz) in enumerate(CT)]
        vep = [W.tile([csz, 3], F32, tag=f"vep{ci}", name=f"vep{ci}")
               for ci, (c0, csz) in enumerate(CT)]
        lt = [W.tile([csz, 3], F32, tag=f"lt{ci}", name=f"lt{ci}")
              for ci, (c0, csz) in enumerate(CT)]
        rstd = [W.tile([csz, 3], F32, tag=f"rstd{ci}", name=f"rstd{ci}")
                for ci, (c0, csz) in enumerate(CT)]
        tmp = [W.tile([csz, 2], F32, tag=f"tmp{ci}", name=f"tmp{ci}")
               for ci, (c0, csz) in enumerate(CT)]
        NTOT_V = float(n_cores * BPC * LK)

        pwqA = [W.tile([128, 192], BF16, tag="pwqA0", name="pwqA0"),
                W.tile([65, 192], BF16, tag="pwqA1", name="pwqA1")]
        pwkA = [W.tile([csz, 192], BF16, tag=f"pwkA{ci}", name=f"pwkA{ci}")
                for ci, (c0, csz) in enumerate(CT)]
        browk = W.tile([1, 192], F32, tag="browk")
        pwvT_bf = [W.tile([csz, 192], BF16, tag=f"pwvTb{ci}", name=f"pwvTb{ci}")
                   for ci, (c0, csz) in enumerate(CT)]
        postvec = W.tile([128, 3], F32, tag="postvec")
        wbar = [W.tile([128, 192], BF16, tag="wbar0", name="wbar0"),
                W.tile([64, 192], BF16, tag="wbar1", name="wbar1")]
        kw, cw = {}, {}
        for g in range(NH):
            kw[(g, 0)] = W.tile([128, 192], BF16, tag=f"kw{g}0", name=f"kw{g}0")
            kw[(g, 1)] = W.tile([65, 192], BF16, tag=f"kw{g}1", name=f"kw{g}1")
            cw[(g, 0)] = W.tile([128, 193], BF16, tag=f"cw{g}0", name=f"cw{g}0")
            cw[(g, 1)] = W.tile([65, 193], BF16, tag=f"cw{g}1", name=f"cw{g}1")

        def ab_chain(paths):
            # A = scale * rsqrt(var+eps), b'' = offset/A - mean; batched Ln
            # pass then batched Exp pass (Ln and Exp are in different ACT
            # table sets -- interleaving would reload tables per op)
            p0, p1 = min(paths), max(paths) + 1
            for ci, (c0, csz) in enumerate(CT):
                for p in paths:
                    if p < 2:  # q, k: local batch stats
                        nc.vector.tensor_scalar(ab[ci][:, 3 + p:4 + p],
                                                mv[p][ci][:, 0:1], 1.0, None, ALU.mult)
                        nc.vector.tensor_scalar(vep[ci][:, p:p + 1], mv[p][ci][:, 1:2],
                                                1.0, EPS, ALU.mult, ALU.add)
                    else:      # v: global stats from the AllReduce
                        inv_n = 1.0 / NTOT_V
                        mean_v = ab[ci][:, 5:6]
                        nc.vector.tensor_scalar(mean_v, gst[ci][:, 0:1], inv_n,
                                                None, ALU.mult)
                        nc.vector.tensor_scalar(tmp[ci][:, 0:1], gst[ci][:, 1:2],
                                                inv_n, EPS, ALU.mult, ALU.add)
                        nc.vector.tensor_scalar(tmp[ci][:, 1:2], mean_v, mean_v,
                                                None, ALU.mult)
                        nc.vector.tensor_tensor(vep[ci][:, 2:3], tmp[ci][:, 0:1],
                                                tmp[ci][:, 1:2], ALU.subtract)
                nc.scalar.activation(lt[ci][:, p0:p1], vep[ci][:, p0:p1], AF.Ln)
            for ci, (c0, csz) in enumerate(CT):
                nc.scalar.activation(rstd[ci][:, p0:p1], lt[ci][:, p0:p1],
                                     AF.Exp, scale=-0.5)
                for p in paths:
                    A = ab[ci][:, p:p + 1]
                    nc.vector.tensor_scalar(A, rstd[ci][:, p:p + 1],
                                            vecs[ci][:, 2 * p:2 * p + 1], None, ALU.mult)
                    recA = tmp[ci][:, 0:1]
                    nc.vector.reciprocal(recA, A)
                    bpp = tmp[ci][:, 1:2]       # b'' = offset*recA - mean
                    nc.vector.scalar_tensor_tensor(bpp, vecs[ci][:, 2 * p + 1:2 * p + 2],
                                                   recA, ab[ci][:, 3 + p:4 + p],
                                                   ALU.mult, ALU.subtract)
                    nc.vector.tensor_scalar(bbf[ci][:, p:p + 1], bpp, 1.0, None, ALU.mult)
                    if p == 0:
                        nc.vector.tensor_scalar(A, A, QK_BAL, None, ALU.mult)

        # ---------- phase 1+2: convs (order v, k, q) with folds interleaved ----------
        with tc.tile_pool(name="pconv", bufs=6, space=MS.PSUM) as PCONV, \
             tc.tile_pool(name="prow", bufs=1, space=MS.PSUM) as PROW, \
             tc.tile_pool(name="pcw", bufs=1, space=MS.PSUM) as PCW:

            def conv_kv(b, p):
                # stride 2 over xkv, psum chunks of 392 (14 output rows);
                # psum->y copies on VectorE (ScalarE is busy staging xq)
                for ci, (c0, csz) in enumerate(CT):
                    xv = xpad[(1, b, ci)].rearrange("p (h th w tw) -> p h th w tw",
                                                    th=2, tw=2, w=29)
                    pss = [PCONV.tile([csz, 392], F32, tag="convps", name="convps")
                           for _ in range(2)]
                    t = 0
                    for dy in (0, 1, 2):
                        for dx in (0, 1, 2):
                            for kc in range(2):
                                h0, th = divmod(28 * kc + dy + 1, 2)
                                w0, tw = divmod(dx + 1, 2)
                                rhs = xv[0:csz, h0:h0 + 14, th, w0:w0 + 28, tw]
                                nc.tensor.matmul(pss[kc][:, :], diag[(p, ci)][:, t, :], rhs,
                                                 start=(t == 0), stop=(t == 8))
                            t += 1
                    for kc in range(2):
                        si = 2 * b + kc
                        ysl = y[(b, p, ci)][0:csz, 392 * kc:392 * (kc + 1)]
                        nc.vector.tensor_copy(ysl, pss[kc][:, :])
                        nc.vector.bn_stats(slots[p][ci][:, 6 * si:6 * si + 6], ysl)

            def conv_q(b):
                # stride 1, psum chunks of 448 (8 output rows); taps outer so
                # runs of 7 matmuls share one diag LDWEIGHTS
                for ci, (c0, csz) in enumerate(CT):
                    xv = xpad[(0, b, ci)].rearrange("p (h w) -> p h w", w=PADW)
                    pss = [PCONV.tile([csz, 448], F32, tag="convps", name="convps")
                           for _ in range(7)]
                    t = 0
                    for dy in (-1, 0, 1):
                        for dx in (-1, 0, 1):
                            for qc in range(7):
                                r0 = 8 * qc + 1 + dy
                                rhs = xv[0:csz, r0:r0 + 8, 1 + dx:57 + dx]
                                nc.tensor.matmul(pss[qc][:, :], diag[(0, ci)][:, t, :], rhs,
                                                 start=(t == 0), stop=(t == 8))
                            t += 1
                    for qc in range(7):
                        si = 7 * b + qc
                        ysl = y[(b, 0, ci)][0:csz, 448 * qc:448 * (qc + 1)]
                        nc.scalar.activation(ysl, pss[qc][:, :], AF.Copy)
                        nc.vector.bn_stats(slots[0][ci][:, 6 * si:6 * si + 6], ysl)

            conv_kv(0, 2)
            conv_kv(1, 2)

            # v stats -> (sum, sumsq) -> AllReduce, launched under the k/q convs
            NLOC_V = float(BPC * LK)
            for ci, (c0, csz) in enumerate(CT):
                nc.vector.bn_aggr(mv[2][ci][:, 0:2], slots[2][ci][:, 0:24])
                m = mv[2][ci][:, 0:1]
                v = mv[2][ci][:, 1:2]
                nc.vector.tensor_scalar(ccst[ci][:, 0:1], m, NLOC_V, None, ALU.mult)
                nc.vector.tensor_scalar(ccst[ci][:, 2:3], m, m, None, ALU.mult)
                nc.vector.tensor_scalar(ccst[ci][:, 1:2], v, ccst[ci][:, 2:3],
                                        NLOC_V, ALU.add, ALU.mult)
                nc.gpsimd.dma_start(cc_in.ap()[c0:c0 + csz, :], ccst[ci][:, 0:2])
            if mock_cc:
                nc.gpsimd.dma_start(cc_out.ap()[:, :], cc_in.ap()[:, :])
            else:
                nc.gpsimd.collective_compute(
                    "AllReduce", ALU.add, replica_groups=[list(range(n_cores))],
                    ins=[cc_in.ap()], outs=[cc_out.ap()])
            for ci, (c0, csz) in enumerate(CT):
                nc.gpsimd.dma_start(gst[ci][:, :], cc_out.ap()[c0:c0 + csz, :])

            conv_kv(0, 1)
            conv_kv(1, 1)
            for ci in range(2):
                nc.vector.bn_aggr(mv[1][ci][:, 0:2], slots[1][ci][:, 0:24])

            conv_q(0)

            # ----- fold k while the PE runs q convs (local stats only, so
            # the PE's kw bias-row matmul can't wedge on the AllReduce) -----
            ab_chain((1,))

            # k weights + pre_sm folding
            for ci, (c0, csz) in enumerate(CT):
                nc.vector.tensor_scalar(pwkA[ci][:, :], pwk_sb[ci][:, :],
                                        ab[ci][:, 1:2], None, ALU.mult)
            rps = PROW.tile([1, 192], F32, tag="rowps", name="rowps")
            nc.tensor.matmul(rps[:, :], bbf[0][:, 1:2], pwkA[0][:, :], start=True, stop=False)
            nc.tensor.matmul(rps[:, :], bbf[1][:, 1:2], pwkA[1][:, :], start=False, stop=True)
            nc.vector.tensor_copy(browk[:, :], rps[:, :])
            for g in range(NH):
                for h in range(NH):
                    col = 3 * h + g
                    for ci, (c0, csz) in enumerate(CT):
                        nc.vector.tensor_scalar(kw[(g, ci)][0:csz, 64 * h:64 * (h + 1)],
                                                pwkA[ci][:, 64 * h:64 * (h + 1)],
                                                smbc[0:csz, col:col + 1], None, ALU.mult)
                    nc.vector.tensor_scalar(kw[(g, 1)][64:65, 64 * h:64 * (h + 1)],
                                            browk[:, 64 * h:64 * (h + 1)],
                                            smbc[0:1, col:col + 1], None, ALU.mult)

            conv_q(1)

            # ----- fold v (needs the AllReduce, long since landed) -----
            ab_chain((2,))

            # v weights: cw_aug_g with post_sm + w_out folded
            for ci, (c0, csz) in enumerate(CT):
                nc.vector.tensor_copy(pwvT_bf[ci][:, :], pwvT_sb[ci][:, :])
            for g in range(NH):
                nc.vector.tensor_copy(postvec[0:64, g:g + 1], smbc[0:64, 9 + 3 * g:10 + 3 * g])
                nc.vector.tensor_copy(postvec[64:128, g:g + 1], smbc[64:128, 10 + 3 * g:11 + 3 * g])
                nc.vector.tensor_scalar(wbar[0][:, :], wout_sb[0][:, :],
                                        postvec[:, g:g + 1], None, ALU.mult)
                nc.vector.tensor_scalar(wbar[1][:, :], wout_sb[1][:, :],
                                        smbc[0:64, 11 + 3 * g:12 + 3 * g], None, ALU.mult)
                for ci, (c0, csz) in enumerate(CT):
                    ps = PCW.tile([csz, 192], F32, tag="cwps", name="cwps")
                    nc.tensor.matmul(ps[:, :], pwvT_bf[0][:, c0:c0 + csz], wbar[0][:, :],
                                     start=True, stop=False)
                    nc.tensor.matmul(ps[:, :], pwvT_bf[1][:, c0:c0 + csz], wbar[1][:, :],
                                     start=False, stop=True)
                    nc.scalar.activation(cw[(g, ci)][0:csz, 0:192], ps[:, :], AF.Copy,
                                         scale=ab[ci][:, 2:3])
                rps = PROW.tile([1, 192], F32, tag="rowps", name="rowps")
                nc.tensor.matmul(rps[:, :], bbf[0][:, 2:3], cw[(g, 0)][0:128, 0:192],
                                 start=True, stop=False)
                nc.tensor.matmul(rps[:, :], bbf[1][:, 2:3], cw[(g, 1)][0:64, 0:192],
                                 start=False, stop=True)
                nc.vector.tensor_copy(cw[(g, 1)][64:65, 0:192], rps[:, :])
                nc.gpsimd.memset(cw[(g, 0)][:, 192:193], 0.0)
                nc.gpsimd.memset(cw[(g, 1)][0:64, 192:193], 0.0)
                nc.gpsimd.memset(cw[(g, 1)][64:65, 192:193], 1.0)

            for ci in range(2):
                nc.vector.bn_aggr(mv[0][ci][:, 0:2], slots[0][ci][:, 0:84])

        # ---------- phase 3: projections + attention ----------
        with tc.tile_pool(name="qtpool", bufs=2) as QT, \
             tc.tile_pool(name="ktpool", bufs=2) as KTP, \
             tc.tile_pool(name="vwpool", bufs=2) as VWP, \
             tc.tile_pool(name="epool", bufs=2) as EP, \
             tc.tile_pool(name="accpool", bufs=3) as ACC, \
             tc.tile_pool(name="outpool", bufs=3) as OUT, \
             tc.tile_pool(name="rzpool", bufs=8) as RZ, \
             tc.tile_pool(name="stri", bufs=2, space=MS.PSUM) as STRI, \
             tc.tile_pool(name="pav", bufs=2, space=MS.PSUM) as PAV:

            for b in range(BPC):
                # q/k projections land in fp8 [96, 2, L]: channel c = 96*i + p,
                # the layout the DoubleRow score matmul contracts directly.
                kt = {}
                for g in range(NH):
                    kt[g] = KTP.tile([96, 2, LK], FP8, tag=f"kt{g}", name=f"kt{g}")
                    for k0 in (0, 392):
                        for mi in range(2):
                            ps3 = STRI.tile([112, 3, 512], F32, tag="stri", name="stri")
                            ps = ps3[0:96, 0, 0:392]
                            nc.tensor.matmul(ps, kw[(g, 0)][:, 96 * mi:96 * mi + 96],
                                             y[(b, 1, 0)][:, k0:k0 + 392], start=True, stop=False)
                            nc.tensor.matmul(ps, kw[(g, 1)][:, 96 * mi:96 * mi + 96],
                                             y[(b, 1, 1)][:, k0:k0 + 392], start=False, stop=True)
                            nc.vector.tensor_copy(kt[g][:, mi, k0:k0 + 392], ps)

                if b == 0:
                    # ----- fold q: its vector/scalar chain runs under the kt
                    # projection above (which only needs kw) -----
                    ab_chain((0,))
                    for ci, (c0, csz) in enumerate(CT):
                        nc.vector.tensor_scalar(pwqA[ci][0:csz, :], pwq_sb[ci][:, :],
                                                ab[ci][:, 0:1], None, ALU.mult)
                    rps = STRI.tile([112, 3, 512], F32, tag="stri",
                                    name="stri")[0:1, 0, 0:192]
                    nc.tensor.matmul(rps, bbf[0][:, 0:1], pwqA[0][0:128, :],
                                     start=True, stop=False)
                    nc.tensor.matmul(rps, bbf[1][:, 0:1], pwqA[1][0:64, :],
                                     start=False, stop=True)
                    nc.vector.tensor_copy(pwqA[1][64:65, :], rps)

                vw = {}
                for g in range(NH):
                    vw[g] = VWP.tile([112, 7, 193], BF16, tag=f"vw{g}", name=f"vw{g}")
                    for ki, (k0, ksz) in enumerate(KT112):
                        ps = PAV.tile([128, 193], F32, tag="avps", name="avps")[0:112, :]
                        nc.tensor.matmul(ps, y[(b, 2, 0)][:, k0:k0 + ksz], cw[(g, 0)][:, :],
                                         start=True, stop=False)
                        nc.tensor.matmul(ps, y[(b, 2, 1)][:, k0:k0 + ksz], cw[(g, 1)][:, :],
                                         start=False, stop=True)
                        nc.vector.tensor_copy(vw[g][:, ki, :], ps)

                qt = QT.tile([96, 2, LQP], FP8, tag="qt", name="qt")
                for mi in range(2):
                    nc.gpsimd.memset(qt[:, mi, LQ:LQP], 0.0)
                for l0, lsz in [(i * 512, 512) for i in range(6)] + [(3072, 64)]:
                    for mi in range(2):
                        ps3 = STRI.tile([112, 3, 512], F32, tag="stri", name="stri")
                        ps = ps3[0:96, 0, 0:lsz]
                        nc.tensor.matmul(ps, pwqA[0][:, 96 * mi:96 * mi + 96],
                                         y[(b, 0, 0)][:, l0:l0 + lsz], start=True, stop=False)
                        nc.tensor.matmul(ps, pwqA[1][:, 96 * mi:96 * mi + 96],
                                         y[(b, 0, 1)][:, l0:l0 + lsz], start=False, stop=True)
                        nc.vector.tensor_copy(qt[:, mi, l0:l0 + lsz], ps)

                for l0, lsz in LCH:
                    ee = {}
                    for g in range(NH):
                        ee[g] = EP.tile([112, 7, 512], BF16, tag=f"e{g}", name=f"e{g}")
                        for g0 in (0, 3, 6):
                            ng = min(3, 7 - g0)
                            ps3 = STRI.tile([112, 3, 512], F32, tag="stri", name="stri")
                            for j in range(ng):
                                k0, ksz = KT112[g0 + j]
                                nc.tensor.matmul(ps3[:, j, 0:lsz], kt[g][:, :, k0:k0 + ksz],
                                                 qt[:, :, l0:l0 + lsz],
                                                 start=True, stop=True, perf_mode=DR)
                            nc.scalar.activation(ee[g][:, g0:g0 + ng, 0:lsz],
                                                 ps3[:, 0:ng, 0:lsz], AF.Exp)

                    for ls in range(0, lsz, 128):
                        acc = ACC.tile([128, 192], F32, tag="acc", name="acc")
                        ot = OUT.tile([128, 192], F32, tag="ot", name="ot")
                        for g in range(NH):
                            ps = PAV.tile([128, 193], F32, tag="avps", name="avps")
                            pv = ps[0:128, 0:193]
                            for ki in range(7):
                                nc.tensor.matmul(pv, ee[g][:, ki, ls:ls + 128],
                                                 vw[g][:, ki, :],
                                                 start=(ki == 0), stop=(ki == 6))
                            r = RZ.tile([128, 1], F32, tag="rz", name="rz")
                            nc.vector.reciprocal(r[:, :], pv[:, 192:193])
                            if g == 0:
                                nc.vector.tensor_scalar(acc[:, :], pv[:, 0:192], r[:, :],
                                                        None, ALU.mult)
                            elif g == 1:
                                nc.vector.scalar_tensor_tensor(acc[:, :], pv[:, 0:192],
                                                               r[:, :], acc[:, :],
                                                               ALU.mult, ALU.add)
                            else:
                                nc.vector.scalar_tensor_tensor(ot[:, :], pv[:, 0:192],
                                                               r[:, :], acc[:, :],
                                                               ALU.mult, ALU.add)
                        lw_out = 64 if l0 + ls == 3072 else 128
                        nc.sync.dma_start(out_d.ap()[b, l0 + ls:l0 + ls + lw_out, :],
                                          ot[0:lw_out, :])


def build(n_cores=N_CORES, mock_cc=False):
    nc = bacc.Bacc("TRN2", target_bir_lowering=False, debug=False, num_devices=n_cores)
    xq = nc.dram_tensor("xq", [BPC, C, PADN], BF16, kind="ExternalInput")
    xkv = nc.dram_tensor("xkv", [BPC, C, PADN], BF16, kind="ExternalInput")
    diag_d = {}
    for p in range(3):
        for ci, (c0, csz) in enumerate(CT):
            diag_d[(p, ci)] = nc.dram_tensor(f"diag{p}{ci}", [csz, 9, csz], BF16,
                                             kind="ExternalInput")
    vecs_d = nc.dram_tensor("vecs", [C, 6], F32, kind="ExternalInput")
    pwq_d = nc.dram_tensor("pwq", [C, HD], F32, kind="ExternalInput")
    pwk_d = nc.dram_tensor("pwk", [C, HD], F32, kind="ExternalInput")
    pwvT_d = nc.dram_tensor("pwvT", [HD, C], F32, kind="ExternalInput")
    wout_d = nc.dram_tensor("wout", [HD, C], F32, kind="ExternalInput")
    sm_d = nc.dram_tensor("smm", [128, 18], F32, kind="ExternalInput")
    out_d = nc.dram_tensor("out", [BPC, LQ, C], F32, kind="ExternalOutput")
    cc_in = nc.dram_tensor("cc_in", [C, 2], F32)
    cc_out = nc.dram_tensor("cc_out", [C, 2], F32)

    io = (xq, xkv, diag_d, vecs_d, pwq_d, pwk_d, pwvT_d, wout_d, sm_d, out_d,
          cc_in, cc_out)
    with tile.TileContext(nc) as tc:
        _emit(tc, nc, io, n_cores=n_cores, mock_cc=mock_cc)
    nc.compile()
    return nc


@functools.lru_cache(maxsize=1)
def _built():
    return build()


def make_in_maps(inputs):
    f32c = lambda a: np.ascontiguousarray(np.asarray(a), dtype=np.float32)
    bf16c = lambda a: np.ascontiguousarray(np.asarray(a, dtype=np.float32)
                                           .astype(ml_dtypes.bfloat16))
    def pad_cm(a):  # [16,56,56,C] -> channel-major zero-padded [16, C, 58*58]
        t = np.asarray(a, dtype=np.float32).transpose(0, 3, 1, 2)
        p = np.zeros((16, C, PADW, PADW), dtype=ml_dtypes.bfloat16)
        p[:, :, 1:57, 1:57] = t.astype(ml_dtypes.bfloat16)
        return np.ascontiguousarray(p.reshape(16, C, PADN))

    xq_t = pad_cm(inputs["inputs_q"])
    xkv_t = pad_cm(inputs["inputs_kv"])
    dwt = np.concatenate([np.asarray(inputs[k], dtype=np.float32).reshape(9, C).T
                          for k in ("dw_q", "dw_k", "dw_v")], axis=1)  # [C, 27]
    base = {
        "vecs": f32c(np.stack([np.asarray(inputs[k]) for k in
                               ("scale_q", "offset_q", "scale_k", "offset_k",
                                "scale_v", "offset_v")], axis=1)),
        "smm": f32c(np.broadcast_to(np.concatenate(
            [np.asarray(inputs["pre_sm"]).reshape(-1) * QK_BAL,
             np.asarray(inputs["post_sm"]).reshape(-1)])[None, :], (128, 18))),
        "pwq": f32c(inputs["pw_q"]), "pwk": f32c(inputs["pw_k"]),
        "pwvT": f32c(np.asarray(inputs["pw_v"]).T), "wout": f32c(inputs["w_out"]),
    }
    for p in range(3):
        for ci, (c0, csz) in enumerate(CT):
            d = np.zeros((csz, 9, csz), dtype=ml_dtypes.bfloat16)
            blk = dwt[c0:c0 + csz, 9 * p:9 * p + 9].astype(ml_dtypes.bfloat16)
            d[np.arange(csz)[:, None], np.arange(9)[None, :], np.arange(csz)[:, None]] = blk
            base[f"diag{p}{ci}"] = d
    in_maps = []
    for i in range(N_CORES):
        m = dict(base)
        m["xq"] = np.ascontiguousarray(xq_t[BPC * i:BPC * (i + 1)])
        m["xkv"] = np.ascontiguousarray(xkv_t[BPC * i:BPC * (i + 1)])
        in_maps.append(m)
    return in_maps


def kernel(**inputs):
    global last_results
    nc = _built()
    in_maps = make_in_maps(inputs)
    trace = os.environ.get("BASS_KERNEL_TRACE", "0") == "1"
    res = run_bass_kernel_spmd(nc, in_maps, core_ids=list(range(N_CORES)), trace=trace)
    last_results = res
    out = np.concatenate([res.results[i]["out"] for i in range(N_CORES)], axis=0)
    return out.astype(np.float32)


if __name__ == "__main__":
    import reference
    inputs = reference.setup_inputs()
    expected = np.asarray(reference.reference(**inputs))
    actual = kernel(**inputs)
    d = np.abs(actual - expected)
    print(f"absmax={d.max():.3e} scale={np.abs(expected).max():.3e} "
          f"rel={d.max() / np.abs(expected).max():.3e}")
